# revision 7
# baseline (speedup 1.0000x reference)
"""GCN encoder (2-layer GCNConv, PyG-style) on 8 Trainium2 NeuronCores.

Sharding: nodes row-sharded 6250/core; edges partitioned by destination-node
owner; per-core segment-sum over 128-dst-slot windows via selection-matrix
matmuls.

Layer 1 is aggregate-then-transform: since the conv is linear before the
nonlinearity, segsum(norm .* (x@W1)[src]) == dinv_d .* segsum(dinv_s .* x[src]) @ W1,
so cores gather dinv.*x rows DIRECTLY from the (host-prepared) input table -
no replicated feature-transform GEMM, no table build on the critical path,
and gathers start at t=0. Per window w the chain is then
    a1  = dinv_d^2 .* segsum1          (scale folded into PSUM evacuation)
    t2  = relu(a1 @ W1) @ W2           (rows of the layer-2 message table)
which is exact for zero biases (as in the reference).

Layer 2: t2 is all-gathered - split into two collectives (sub-tables A/B)
that overlap with remaining gather/compute work - then aggregated the same
way; out = dinv_d .* (segsum2 + t2_own).

Self-loop messages never go through the gather path: their contribution to a
window's segment-sum is one identity matmul from an SBUF-resident copy of the
core's own rows.

Sub-tables (for int16 gather indices and collective splitting): local row
l < 3200 (windows 0-24) -> sub A (8*3200 = 25600 rows); l >= 3200
(windows 25-48) -> sub B (8*3072 = 24576 rows). Both < 2**15.
"""

import os
import numpy as np
import ml_dtypes

import concourse.bacc as bacc
import concourse.tile as tile
from concourse import bass, mybir
from concourse.bass_utils import run_bass_kernel_spmd
from concourse.library_config import mlp

N = 50000
INC, HID, OUTC = 256, 256, 128
NCORES = 8
RPC = N // NCORES            # 6250 rows per core
WPC = (RPC + 127) // 128     # 49 windows per core
RPAD = WPC * 128             # 6272
LSPL = 3200                  # sub-table split on local row (windows 0..24 | 25..48)
NA = NCORES * LSPL           # 25600 rows in sub-table A
NB = NCORES * (RPAD - LSPL)  # 24576 rows in sub-table B
WA = LSPL // 128             # 25 windows in A
GRP = 2                      # windows per supergather group
NGRP = (WPC + GRP - 1) // GRP
# L1 processes B-side groups first so AG2(B) can launch early.
# group NGA = WA//GRP straddles the A/B boundary (windows 24,25) and is
# processed in the B phase, so after the B phase windows 24..48 are all done.
NGA = WA // GRP
GORDER = list(range(NGA, NGRP)) + list(range(0, NGA))


def _preprocess(edge_index):
    """Edge partitioning / ordering and normalization constants (host, index-only)."""
    src = np.asarray(edge_index[0], np.int64)
    dst = np.asarray(edge_index[1], np.int64)

    # degrees include the self-loops the reference adds
    deg = (np.bincount(dst, minlength=N) + 1).astype(np.float64)
    dinv = (1.0 / np.sqrt(deg)).astype(np.float32)

    owner = dst // RPC
    dstl = dst - owner * RPC
    win = dstl >> 7
    slot = dstl & 127
    srho = src // RPC
    srl = src - srho * RPC
    sub = (srl >= LSPL).astype(np.int64)
    gl = np.where(sub == 0, srho * LSPL + srl,
                  srho * (RPAD - LSPL) + (srl - LSPL)).astype(np.int32)

    key = (owner * WPC + win) * 2 + sub
    order = np.argsort(key, kind="stable")
    key_s = key[order]
    gl_s = gl[order]
    slot_s = slot[order].astype(np.int32)

    nbuck = NCORES * WPC * 2
    counts = np.bincount(key_s, minlength=nbuck).reshape(NCORES, WPC, 2)
    starts_flat = np.concatenate([[0], np.cumsum(counts.reshape(-1))])

    # tiles per (window, sub): max over cores so one SPMD program fits all
    Twh = (counts.max(axis=0) + 127) // 128     # [WPC, 2]
    TT = int(Twh.sum())
    # stream order: group -> sub -> window in group -> tiles
    base = np.zeros((WPC, 2), np.int64)
    pos = 0
    for gi in range(NGRP):
        ws = range(gi * GRP, min((gi + 1) * GRP, WPC))
        for h in range(2):
            for w in ws:
                base[w, h] = pos
                pos += Twh[w, h]
    assert pos == TT

    idx_seq = np.zeros((NCORES, TT * 128), np.int32)
    slot_seq = np.full((NCORES, TT * 128), 128, np.int32)  # 128 = dropped sentinel
    for c in range(NCORES):
        for w in range(WPC):
            for h in range(2):
                n = counts[c, w, h]
                if n == 0:
                    continue
                s0 = starts_flat[(c * WPC + w) * 2 + h]
                p0 = base[w, h] * 128
                idx_seq[c, p0 : p0 + n] = gl_s[s0 : s0 + n]
                slot_seq[c, p0 : p0 + n] = slot_s[s0 : s0 + n]

    # wrapped int16 gather-index layout: element j at [j%16, j//16], replicated x8
    idx16 = np.empty((NCORES, 128, TT * 8), np.int16)
    slots = np.empty((NCORES, 128, TT), ml_dtypes.bfloat16)
    for c in range(NCORES):
        a = idx_seq[c].astype(np.int16).reshape(-1, 16).T
        idx16[c] = np.tile(a, (8, 1))
        slots[c] = slot_seq[c].astype(ml_dtypes.bfloat16).reshape(TT, 128).T

    # per-core per-window dinv columns for own rows
    dcol1 = np.zeros((NCORES, 128, WPC), np.float32)
    for c in range(NCORES):
        d = np.zeros(RPAD, np.float32)
        d[:RPC] = dinv[c * RPC : (c + 1) * RPC]
        dcol1[c] = d.reshape(WPC, 128).T
    dcol2 = dcol1 * dcol1

    return idx16, slots, Twh, base, TT, dcol1, dcol2, dinv


def _xg_table(x, dinv):
    """dinv .* x rows in [A | B] rank-major padded order, bf16 (the L1 gather table)."""
    xd = (x * dinv[:, None]).astype(np.float32)
    xg = np.zeros((NA + NB, INC), np.float32)
    nb = RPAD - LSPL
    for rho in range(NCORES):
        xs = xd[rho * RPC : (rho + 1) * RPC]         # [6250, 256]
        xg[rho * LSPL : (rho + 1) * LSPL] = xs[:LSPL]
        xg[NA + rho * nb : NA + rho * nb + (RPC - LSPL)] = xs[LSPL:]
    return np.ascontiguousarray(xg).astype(ml_dtypes.bfloat16)


def _build(TT, Twh, base):
    # 32KB/partition descriptor scratch: the default 16KB holds only ~3.5
    # gathers' descriptors, so a 4th in-flight gather stalls the Pool engine
    # waiting for ring space.
    nc = bacc.Bacc("TRN2", num_devices=NCORES, num_swdge_queues=4,
                   dynamic_dma_scratch_size=32768)
    f32 = mybir.dt.float32
    bf = mybir.dt.bfloat16

    xg_d = nc.dram_tensor("xg", [NA + NB, INC], bf, kind="ExternalInput")
    xo_d = nc.dram_tensor("xo", [128, WPC, INC], bf, kind="ExternalInput")
    w1_d = nc.dram_tensor("w1", [2, 128, HID], bf, kind="ExternalInput")
    w2_d = nc.dram_tensor("w2", [2, 128, OUTC], bf, kind="ExternalInput")
    iota_d = nc.dram_tensor("iota", [128, 128], bf, kind="ExternalInput")
    ident_d = nc.dram_tensor("ident", [128, 128], bf, kind="ExternalInput")
    dc1_d = nc.dram_tensor("dcol1", [128, WPC], f32, kind="ExternalInput")
    dc2_d = nc.dram_tensor("dcol2", [128, WPC], f32, kind="ExternalInput")
    idx_d = nc.dram_tensor("idx", [128, TT * 8], mybir.dt.int16, kind="ExternalInput")
    slots_d = nc.dram_tensor("slots", [128, TT], bf, kind="ExternalInput")
    out_d = nc.dram_tensor("out", [RPAD, OUTC], f32, kind="ExternalOutput")

    # tiles per supergather (group, sub)
    Tg = np.zeros((NGRP, 2), np.int64)
    for gi in range(NGRP):
        ws = range(gi * GRP, min((gi + 1) * GRP, WPC))
        for h in range(2):
            Tg[gi, h] = sum(int(Twh[w, h]) for w in ws)

    with tile.TileContext(nc) as tc:
        nc.gpsimd.load_library(mlp)
        with (
            tc.tile_pool(name="const", bufs=1) as cpool,
            tc.tile_pool(name="gt", bufs=1) as gtpool,
            tc.tile_pool(name="evac", bufs=4) as epool,
            tc.tile_pool(name="tsp", bufs=6) as tpool,
            tc.tile_pool(name="msg", bufs=8) as mpool,
            tc.tile_pool(name="sel", bufs=6) as spool,
            tc.tile_pool(name="part", bufs=WPC) as ppool,
            tc.tile_pool(name="p256", bufs=4, space="PSUM") as p256,
            tc.tile_pool(name="p128", bufs=2, space="PSUM") as p128,
            tc.tile_pool(name="ptr", bufs=2, space="PSUM") as ptr,
            tc.tile_pool(name="dram", bufs=1, space="DRAM") as dram,
        ):
            # ---- constants to SBUF
            w1_s = cpool.tile([128, 2, HID], bf)
            w2_s = cpool.tile([128, 2, OUTC], bf)
            iota_s = cpool.tile([128, 128], bf)
            ident_s = cpool.tile([128, 128], bf)
            dc1_s = cpool.tile([128, WPC], f32)
            dc2_s = cpool.tile([128, WPC], f32)
            idx_s = cpool.tile([128, TT * 8], mybir.dt.int16)
            slots_s = cpool.tile([128, TT], bf)
            xo_s = gtpool.tile([128, WPC, INC], bf)     # own dinv.*x rows
            own2_s = gtpool.tile([128, WPC, OUTC], bf)  # own table2 rows
            nc.sync.dma_start(idx_s[:], idx_d[:])
            nc.sync.dma_start(slots_s[:], slots_d[:])
            for k in range(2):
                nc.sync.dma_start(w1_s[:, k, :], w1_d[k])
                nc.sync.dma_start(w2_s[:, k, :], w2_d[k])
            nc.sync.dma_start(iota_s[:], iota_d[:])
            nc.sync.dma_start(ident_s[:], ident_d[:])
            nc.sync.dma_start(dc1_s[:], dc1_d[:])
            nc.sync.dma_start(dc2_s[:], dc2_d[:])
            nc.scalar.dma_start(xo_s[:], xo_d[:])

            ag2a_in = dram.tile([LSPL, OUTC], bf)
            ag2b_in = dram.tile([RPAD - LSPL, OUTC], bf)
            tb2a = dram.tile([NA, OUTC], bf)
            tb2b = dram.tile([NB, OUTC], bf)

            # ---- edge aggregation unit: gather + S build for one (group, sub)
            def gather_unit(gi, h, tbl_ap, width, qctr):
                T = int(Tg[gi, h])
                if T == 0:
                    return None, None
                ws = list(range(gi * GRP, min((gi + 1) * GRP, WPC)))
                b = int(base[ws[0], h])
                m_s = mpool.tile([128, T, width], bf, tag="msg")
                nc.gpsimd.dma_gather(
                    m_s[:], tbl_ap, idx_s[:, b * 8 : (b + T) * 8],
                    T * 128, T * 128, width,
                    single_packet=False, queue_num=qctr[0] % 4)
                qctr[0] += 1
                S_s = spool.tile([128, T, 128], bf, tag="sel")
                nc.vector.tensor_tensor(
                    out=S_s[:],
                    in0=slots_s[:, b : b + T, None].to_broadcast([128, T, 128]),
                    in1=iota_s[:, None, :].to_broadcast([128, T, 128]),
                    op=mybir.AluOpType.is_equal)
                return m_s, S_s

            def win_mms(w, h, ps, m_s, S_s, first, last):
                # base of this unit's stream is base[first window of group, h]
                gw0 = (w // GRP) * GRP
                b = int(base[gw0, h])
                n = int(Twh[w, h])
                for t in range(n):
                    tt = int(base[w, h]) - b + t
                    nc.tensor.matmul(ps[:], lhsT=S_s[:, tt, :], rhs=m_s[:, tt, :],
                                     start=(first and t == 0),
                                     stop=(last and t == n - 1))

            qctr = [0]

            # ---- layer-1 aggregation + per-window transform (B-side groups first)
            def l1_group(gi):
                ws = list(range(gi * GRP, min((gi + 1) * GRP, WPC)))
                units = {}
                for h in range(2):
                    units[h] = gather_unit(
                        gi, h, xg_d[:NA, :] if h == 0 else xg_d[NA:, :], INC, qctr)
                pss = {}
                for w in ws:
                    ps = p256.tile([128, HID], f32, tag="p256")
                    pss[w] = ps
                    last_h = max((h for h in range(2)
                                  if units[h][0] is not None and Twh[w, h] > 0),
                                 default=None)
                    # self-loop contribution first (operands ready at t=0)
                    nc.tensor.matmul(ps[:], lhsT=ident_s[:], rhs=xo_s[:, w, :],
                                     start=True, stop=last_h is None)
                    for h in range(2):
                        m_s, S_s = units[h]
                        if m_s is None or Twh[w, h] == 0:
                            continue
                        win_mms(w, h, ps, m_s, S_s, False, h == last_h)
                for w in ws:
                    ps = pss[w]
                    # a1 = dinv^2 .* segsum1
                    a1 = epool.tile([128, HID], bf, tag="a1")
                    nc.scalar.activation(a1[:], ps[:],
                                         mybir.ActivationFunctionType.Copy,
                                         scale=dc2_s[:, w : w + 1])
                    # u = a1 @ W1  (via 2 transposes of a1)
                    u_ps = p256.tile([128, HID], f32, tag="p256")
                    for k in range(2):
                        pt = ptr.tile([128, 128], bf, tag="pt")
                        nc.tensor.transpose(pt[:], a1[:, k * 128 : (k + 1) * 128],
                                            ident_s[:])
                        a1t = tpool.tile([128, 128], bf, tag="a1t")
                        nc.vector.tensor_copy(a1t[:], pt[:])
                        nc.tensor.matmul(u_ps[:], lhsT=a1t[:], rhs=w1_s[:, k, :],
                                         start=(k == 0), stop=(k == 1))
                    # t2 = relu(u) @ W2
                    v = epool.tile([128, HID], bf, tag="v")
                    nc.scalar.activation(v[:], u_ps[:],
                                         mybir.ActivationFunctionType.Relu)
                    t2_ps = p128.tile([128, OUTC], f32, tag="p128")
                    for k in range(2):
                        pt = ptr.tile([128, 128], bf, tag="pt")
                        nc.tensor.transpose(pt[:], v[:, k * 128 : (k + 1) * 128],
                                            ident_s[:])
                        vt = tpool.tile([128, 128], bf, tag="vt")
                        nc.vector.tensor_copy(vt[:], pt[:])
                        nc.tensor.matmul(t2_ps[:], lhsT=vt[:], rhs=w2_s[:, k, :],
                                         start=(k == 0), stop=(k == 1))
                    nc.vector.tensor_copy(own2_s[:, w, :], t2_ps[:])
                    if w < WA:
                        nc.sync.dma_start(ag2a_in[w * 128 : (w + 1) * 128, :],
                                          own2_s[:, w, :])
                    else:
                        nc.sync.dma_start(ag2b_in[(w - WA) * 128 : (w - WA + 1) * 128, :],
                                          own2_s[:, w, :])

            with nc.named_scope("p3_l1b"):
                for gi in GORDER[: NGRP - NGA]:
                    l1_group(gi)
            # AG2 for sub-table B launches while L1 still works on A-side groups
            with nc.named_scope("ag2b"):
                nc.gpsimd.collective_compute(
                    "AllGather", mybir.AluOpType.bypass,
                    replica_groups=[list(range(NCORES))],
                    ins=[ag2b_in.opt()], outs=[tb2b.opt()])
            with nc.named_scope("p3_l1a"):
                for gi in GORDER[NGRP - NGA :]:
                    l1_group(gi)
            with nc.named_scope("ag2a"):
                nc.gpsimd.collective_compute(
                    "AllGather", mybir.AluOpType.bypass,
                    replica_groups=[list(range(NCORES))],
                    ins=[ag2a_in.opt()], outs=[tb2a.opt()])

            # ---- layer-2 aggregation, two stages so AG latency hides
            partials = {}
            with nc.named_scope("p6_b"):
                # stage 1: self + sub-B messages -> partial (frees PSUM quickly)
                for gi in range(NGRP):
                    ws = list(range(gi * GRP, min((gi + 1) * GRP, WPC)))
                    m_s, S_s = gather_unit(gi, 1, tb2b[:, :], OUTC, qctr)
                    for w in ws:
                        ps = p256.tile([128, OUTC], f32, tag="p256")
                        started = False
                        if m_s is not None and Twh[w, 1] > 0:
                            win_mms(w, 1, ps, m_s, S_s, True, False)
                            started = True
                        nc.tensor.matmul(ps[:], lhsT=ident_s[:], rhs=own2_s[:, w, :],
                                         start=not started, stop=True)
                        pp = ppool.tile([128, OUTC], bf, tag="partial")
                        nc.vector.tensor_copy(pp[:], ps[:])
                        partials[w] = pp
            with nc.named_scope("p6_a"):
                # stage 2: sub-A messages + partial -> output
                for gi in range(NGRP):
                    ws = list(range(gi * GRP, min((gi + 1) * GRP, WPC)))
                    m_s, S_s = gather_unit(gi, 0, tb2a[:, :], OUTC, qctr)
                    for w in ws:
                        o_s = epool.tile([128, OUTC], f32, tag="o")
                        if m_s is not None and Twh[w, 0] > 0:
                            ps = p256.tile([128, OUTC], f32, tag="p256")
                            win_mms(w, 0, ps, m_s, S_s, True, True)
                            acc = epool.tile([128, OUTC], f32, tag="acc")
                            nc.vector.tensor_add(acc[:], ps[:], partials[w][:])
                        else:
                            acc = partials[w]
                        nc.scalar.activation(o_s[:], acc[:],
                                             mybir.ActivationFunctionType.Copy,
                                             scale=dc1_s[:, w : w + 1])
                        nc.sync.dma_start(out_d[w * 128 : (w + 1) * 128, :], o_s[:])

    nc.compile()
    return nc


def kernel(x, edge_index, W1, b1, W2, b2):
    x = np.asarray(x, np.float32)
    W1 = np.asarray(W1, np.float32)
    W2 = np.asarray(W2, np.float32)
    assert not np.any(np.asarray(b1)) and not np.any(np.asarray(b2)), \
        "kernel assumes zero biases (as in the reference setup)"

    idx16, slots, Twh, base, TT, dcol1, dcol2, dinv = _preprocess(np.asarray(edge_index))
    nc = _build(TT, Twh, base)

    iota = np.broadcast_to(np.arange(128, dtype=np.float32), (128, 128)).astype(ml_dtypes.bfloat16)
    ident = np.eye(128, dtype=np.float32).astype(ml_dtypes.bfloat16)
    w1_in = np.ascontiguousarray(W1.reshape(2, 128, HID)).astype(ml_dtypes.bfloat16)
    w2_in = np.ascontiguousarray(W2.reshape(2, 128, OUTC)).astype(ml_dtypes.bfloat16)
    xg = _xg_table(x, dinv)

    xd = (x * dinv[:, None]).astype(np.float32)
    in_maps = []
    for c in range(NCORES):
        xo = np.zeros((RPAD, INC), np.float32)
        xo[:RPC] = xd[c * RPC : (c + 1) * RPC]
        xo = np.ascontiguousarray(
            xo.reshape(WPC, 128, INC).transpose(1, 0, 2)).astype(ml_dtypes.bfloat16)
        in_maps.append({
            "xg": xg, "xo": xo,
            "w1": w1_in, "w2": w2_in, "iota": iota, "ident": ident,
            "dcol1": dcol1[c], "dcol2": dcol2[c],
            "idx": idx16[c], "slots": slots[c],
        })

    trace = bool(int(os.environ.get("GCN_KERNEL_TRACE", "0")))
    try:
        res = run_bass_kernel_spmd(nc, in_maps, core_ids=list(range(NCORES)), trace=trace)
    except Exception:
        # rare transient NRT exec failure: retry once on a fresh dispatch
        time_mod = __import__("time"); time_mod.sleep(2.0)
        res = run_bass_kernel_spmd(nc, in_maps, core_ids=list(range(NCORES)), trace=False)
    kernel.last_results = res
    if trace:
        print(f"HW exec time: {res.exec_time_ns} ns")
        kernel.last_exec_time_ns = res.exec_time_ns

    out = np.concatenate([res.results[c]["out"][:RPC] for c in range(NCORES)], axis=0)
    return out.astype(np.float32)


# revision 20
# speedup vs baseline: 1.0260x; 1.0260x over previous
"""GCN encoder (2-layer GCNConv, PyG-style) on 8 Trainium2 NeuronCores.

Sharding: nodes row-sharded 6250/core; edges partitioned by destination-node
owner; per-core segment-sum over 128-dst-slot windows via selection-matrix
matmuls.

Layer 1 is aggregate-then-transform: since the conv is linear before the
nonlinearity, segsum(norm .* (x@W1)[src]) == dinv_d .* segsum(dinv_s .* x[src]) @ W1,
so cores gather dinv.*x rows DIRECTLY from the (host-prepared) input table -
no replicated feature-transform GEMM, no table build on the critical path,
and gathers start at t=0. Per window w the chain is then
    a1  = dinv_d^2 .* segsum1          (scale folded into PSUM evacuation)
    t2  = relu(a1 @ W1) @ W2           (rows of the layer-2 message table)
which is exact for zero biases (as in the reference).

Layer 2: t2 is all-gathered - split into two collectives (sub-tables A/B)
that overlap with remaining gather/compute work - then aggregated the same
way; out = dinv_d .* (segsum2 + t2_own).

Self-loop messages never go through the gather path: their contribution to a
window's segment-sum is one identity matmul from an SBUF-resident copy of the
core's own rows.

Sub-tables (for int16 gather indices and collective splitting): local row
l < 3200 (windows 0-24) -> sub A (8*3200 = 25600 rows); l >= 3200
(windows 25-48) -> sub B (8*3072 = 24576 rows). Both < 2**15.
"""

import os
import numpy as np
import ml_dtypes

import concourse.bacc as bacc
import concourse.tile as tile
from concourse import bass, mybir
from concourse.bass_utils import run_bass_kernel_spmd
from concourse.library_config import mlp

N = 50000
INC, HID, OUTC = 256, 256, 128
NCORES = 8
RPC = N // NCORES            # 6250 rows per core
WPC = (RPC + 127) // 128     # 49 windows per core
RPAD = WPC * 128             # 6272
LSPL = 3200                  # sub-table split on local row (windows 0..24 | 25..48)
NA = NCORES * LSPL           # 25600 rows in sub-table A
NB = NCORES * (RPAD - LSPL)  # 24576 rows in sub-table B
WA = LSPL // 128             # 25 windows in A
GRP = 1                      # windows per supergather group
NGRP = (WPC + GRP - 1) // GRP
# L1 processes B-side groups first so AG2(B) can launch early.
# group NGA = WA//GRP straddles the A/B boundary (windows 24,25) and is
# processed in the B phase, so after the B phase windows 24..48 are all done.
NGA = WA // GRP
GORDER = list(range(NGA, NGRP)) + list(range(0, NGA))


def _preprocess(edge_index):
    """Edge partitioning / ordering and normalization constants (host, index-only)."""
    src = np.asarray(edge_index[0], np.int64)
    dst = np.asarray(edge_index[1], np.int64)

    # degrees include the self-loops the reference adds
    deg = (np.bincount(dst, minlength=N) + 1).astype(np.float64)
    dinv = (1.0 / np.sqrt(deg)).astype(np.float32)

    owner = dst // RPC
    dstl = dst - owner * RPC
    win = dstl >> 7
    slot = dstl & 127
    srho = src // RPC
    srl = src - srho * RPC
    sub = (srl >= LSPL).astype(np.int64)
    gl = np.where(sub == 0, srho * LSPL + srl,
                  srho * (RPAD - LSPL) + (srl - LSPL)).astype(np.int32)

    key = (owner * WPC + win) * 2 + sub
    order = np.argsort(key, kind="stable")
    key_s = key[order]
    gl_s = gl[order]
    slot_s = slot[order].astype(np.int32)

    nbuck = NCORES * WPC * 2
    counts = np.bincount(key_s, minlength=nbuck).reshape(NCORES, WPC, 2)
    starts_flat = np.concatenate([[0], np.cumsum(counts.reshape(-1))])

    # tiles per (window, sub): max over cores so one SPMD program fits all
    Twh = (counts.max(axis=0) + 127) // 128     # [WPC, 2]
    TT = int(Twh.sum())
    # stream order: group -> sub -> window in group -> tiles
    base = np.zeros((WPC, 2), np.int64)
    pos = 0
    for gi in range(NGRP):
        ws = range(gi * GRP, min((gi + 1) * GRP, WPC))
        for h in range(2):
            for w in ws:
                base[w, h] = pos
                pos += Twh[w, h]
    assert pos == TT

    idx_seq = np.zeros((NCORES, TT * 128), np.int32)
    slot_seq = np.full((NCORES, TT * 128), 128, np.int32)  # 128 = dropped sentinel
    for c in range(NCORES):
        for w in range(WPC):
            for h in range(2):
                n = counts[c, w, h]
                if n == 0:
                    continue
                s0 = starts_flat[(c * WPC + w) * 2 + h]
                p0 = base[w, h] * 128
                idx_seq[c, p0 : p0 + n] = gl_s[s0 : s0 + n]
                slot_seq[c, p0 : p0 + n] = slot_s[s0 : s0 + n]

    # wrapped int16 gather-index layout: element j at [j%16, j//16], replicated x8
    idx16 = np.empty((NCORES, 128, TT * 8), np.int16)
    slots = np.empty((NCORES, 128, TT), ml_dtypes.bfloat16)
    for c in range(NCORES):
        a = idx_seq[c].astype(np.int16).reshape(-1, 16).T
        idx16[c] = np.tile(a, (8, 1))
        slots[c] = slot_seq[c].astype(ml_dtypes.bfloat16).reshape(TT, 128).T

    # per-core per-window dinv columns for own rows
    dcol1 = np.zeros((NCORES, 128, WPC), np.float32)
    for c in range(NCORES):
        d = np.zeros(RPAD, np.float32)
        d[:RPC] = dinv[c * RPC : (c + 1) * RPC]
        dcol1[c] = d.reshape(WPC, 128).T
    dcol2 = dcol1 * dcol1

    return idx16, slots, Twh, base, TT, dcol1, dcol2, dinv


def _xg_table(x, dinv):
    """dinv .* x rows in [A | B] rank-major padded order, bf16 (the L1 gather table)."""
    xd = (x * dinv[:, None]).astype(np.float32)
    xg = np.zeros((NA + NB, INC), np.float32)
    nb = RPAD - LSPL
    for rho in range(NCORES):
        xs = xd[rho * RPC : (rho + 1) * RPC]         # [6250, 256]
        xg[rho * LSPL : (rho + 1) * LSPL] = xs[:LSPL]
        xg[NA + rho * nb : NA + rho * nb + (RPC - LSPL)] = xs[LSPL:]
    return np.ascontiguousarray(xg).astype(ml_dtypes.bfloat16)


def _build(TT, Twh, base):
    # 32KB/partition descriptor scratch: the default 16KB holds only ~3.5
    # gathers' descriptors, so a 4th in-flight gather stalls the Pool engine
    # waiting for ring space.
    nc = bacc.Bacc("TRN2", num_devices=NCORES, num_swdge_queues=4,
                   dynamic_dma_scratch_size=32768)
    f32 = mybir.dt.float32
    bf = mybir.dt.bfloat16

    xg_d = nc.dram_tensor("xg", [NA + NB, INC], bf, kind="ExternalInput")
    xo_d = nc.dram_tensor("xo", [128, WPC, INC], bf, kind="ExternalInput")
    w1b_d = nc.dram_tensor("w1b", [2, 2, 128, 128], bf, kind="ExternalInput")
    w2_d = nc.dram_tensor("w2", [2, 128, OUTC], bf, kind="ExternalInput")
    iota_d = nc.dram_tensor("iota", [128, 128], bf, kind="ExternalInput")
    ident_d = nc.dram_tensor("ident", [128, 128], bf, kind="ExternalInput")
    dc1_d = nc.dram_tensor("dcol1", [128, WPC], f32, kind="ExternalInput")
    dc2_d = nc.dram_tensor("dcol2", [128, WPC], f32, kind="ExternalInput")
    idx_d = nc.dram_tensor("idx", [128, TT * 8], mybir.dt.int16, kind="ExternalInput")
    slots_d = nc.dram_tensor("slots", [128, TT], bf, kind="ExternalInput")
    out_d = nc.dram_tensor("out", [RPAD, OUTC], f32, kind="ExternalOutput")

    # tiles per supergather (group, sub)
    Tg = np.zeros((NGRP, 2), np.int64)
    for gi in range(NGRP):
        ws = range(gi * GRP, min((gi + 1) * GRP, WPC))
        for h in range(2):
            Tg[gi, h] = sum(int(Twh[w, h]) for w in ws)

    with tile.TileContext(nc) as tc:
        nc.gpsimd.load_library(mlp)
        with (
            tc.tile_pool(name="const", bufs=1) as cpool,
            tc.tile_pool(name="gt", bufs=1) as gtpool,
            tc.tile_pool(name="evac", bufs=4) as epool,
            tc.tile_pool(name="tsp", bufs=6) as tpool,
            tc.tile_pool(name="msg", bufs=12) as mpool,
            tc.tile_pool(name="sel", bufs=8) as spool,
            tc.tile_pool(name="part", bufs=WPC) as ppool,
            # PSUM is 8 banks; every buffer costs a full bank
            tc.tile_pool(name="ps1", bufs=3, space="PSUM") as pp1,
            tc.tile_pool(name="pu", bufs=2, space="PSUM") as pu,
            tc.tile_pool(name="pt2", bufs=1, space="PSUM") as pt2,
            tc.tile_pool(name="ptr", bufs=2, space="PSUM") as ptr,
            tc.tile_pool(name="dram", bufs=1, space="DRAM") as dram,
        ):
            # ---- constants to SBUF
            w1b_s = cpool.tile([128, 2, 2, 128], bf)    # W1 as [ch_k][o_j] blocks
            w2_s = cpool.tile([128, 2, OUTC], bf)
            iota_s = cpool.tile([128, 128], bf)
            ident_s = cpool.tile([128, 128], bf)
            dc1_s = cpool.tile([128, WPC], f32)
            dc2_s = cpool.tile([128, WPC], f32)
            idx_s = cpool.tile([128, TT * 8], mybir.dt.int16)
            slots_s = cpool.tile([128, TT], bf)
            xo_s = gtpool.tile([128, WPC, INC], bf)     # own dinv.*x rows
            own2_s = gtpool.tile([128, WPC, OUTC], bf)  # own table2 rows
            nc.sync.dma_start(idx_s[:], idx_d[:])
            nc.sync.dma_start(slots_s[:], slots_d[:])
            for k in range(2):
                for j in range(2):
                    nc.sync.dma_start(w1b_s[:, k, j, :], w1b_d[k, j])
                nc.sync.dma_start(w2_s[:, k, :], w2_d[k])
            nc.sync.dma_start(iota_s[:], iota_d[:])
            nc.sync.dma_start(ident_s[:], ident_d[:])
            nc.sync.dma_start(dc1_s[:], dc1_d[:])
            nc.sync.dma_start(dc2_s[:], dc2_d[:])
            nc.scalar.dma_start(xo_s[:], xo_d[:])

            # zero the msg ring buffers once so padding-skipped rows are finite
            TMAX = int(Tg.max())
            for _ in range(12):
                mz = mpool.tile([128, TMAX, INC], bf, tag="msg")
                nc.vector.memset(mz[:], 0)

            ag2a_in = dram.tile([LSPL, OUTC], bf)
            ag2b_in = dram.tile([RPAD - LSPL, OUTC], bf)
            tb2a = dram.tile([NA, OUTC], bf)
            tb2b = dram.tile([NB, OUTC], bf)

            # ---- edge aggregation unit: gather + S build for one (group, sub)
            def gather_unit(gi, h, tbl_ap, width, qctr):
                T = int(Tg[gi, h])
                if T == 0:
                    return None, None
                ws = list(range(gi * GRP, min((gi + 1) * GRP, WPC)))
                b = int(base[ws[0], h])
                m_s = mpool.tile([128, T, width], bf, tag="msg")
                nc.gpsimd.dma_gather(
                    m_s[:], tbl_ap, idx_s[:, b * 8 : (b + T) * 8],
                    T * 128, T * 128, width,
                    single_packet=False, queue_num=qctr[0] % 4)
                qctr[0] += 1
                S_s = spool.tile([128, T, 128], bf, tag="sel")
                nc.vector.tensor_tensor(
                    out=S_s[:],
                    in0=slots_s[:, b : b + T, None].to_broadcast([128, T, 128]),
                    in1=iota_s[:, None, :].to_broadcast([128, T, 128]),
                    op=mybir.AluOpType.is_equal)
                return m_s, S_s

            def win_mms(w, h, ps, m_s, S_s, first, last):
                # base of this unit's stream is base[first window of group, h]
                gw0 = (w // GRP) * GRP
                b = int(base[gw0, h])
                n = int(Twh[w, h])
                for t in range(n):
                    tt = int(base[w, h]) - b + t
                    nc.tensor.matmul(ps[:], lhsT=S_s[:, tt, :], rhs=m_s[:, tt, :],
                                     start=(first and t == 0),
                                     stop=(last and t == n - 1))

            qctr = [0]

            # ---- layer-1 aggregation + per-window transform (B-side groups first)
            def l1_group(gi):
                ws = list(range(gi * GRP, min((gi + 1) * GRP, WPC)))
                units = {}
                for h in range(2):
                    units[h] = gather_unit(
                        gi, h, xg_d[:NA, :] if h == 0 else xg_d[NA:, :], INC, qctr)
                pss = {}
                for w in ws:
                    ps = pp1.tile([128, HID], f32, tag="ps1")
                    pss[w] = ps
                    last_h = max((h for h in range(2)
                                  if units[h][0] is not None and Twh[w, h] > 0),
                                 default=None)
                    # self-loop contribution first (operands ready at t=0)
                    nc.tensor.matmul(ps[:], lhsT=ident_s[:], rhs=xo_s[:, w, :],
                                     start=True, stop=last_h is None)
                    for h in range(2):
                        m_s, S_s = units[h]
                        if m_s is None or Twh[w, h] == 0:
                            continue
                        win_mms(w, h, ps, m_s, S_s, False, h == last_h)
                for w in ws:
                    ps = pss[w]
                    # a1 = dinv^2 .* segsum1
                    a1 = epool.tile([128, HID], bf, tag="a1")
                    nc.scalar.activation(a1[:], ps[:],
                                         mybir.ActivationFunctionType.Copy,
                                         scale=dc2_s[:, w : w + 1])
                    # a1^T via 2 PE transposes
                    a1t = tpool.tile([128, 2, 128], bf, tag="a1t")
                    for k in range(2):
                        pt = ptr.tile([128, 128], bf, tag="pt")
                        nc.tensor.transpose(pt[:], a1[:, k * 128 : (k + 1) * 128],
                                            ident_s[:])
                        nc.vector.tensor_copy(a1t[:, k, :], pt[:])
                    # u^T = (a1 @ W1)^T directly: lhsT = W1 block, rhs = a1^T
                    uT = pu.tile([128, 2, 128], f32, tag="uT")
                    for j in range(2):
                        for k in range(2):
                            nc.tensor.matmul(uT[:, j, :], lhsT=w1b_s[:, k, j, :],
                                             rhs=a1t[:, k, :],
                                             start=(k == 0), stop=(k == 1))
                    # v^T = relu(u^T) is directly the lhsT for W2
                    vt = tpool.tile([128, 2, 128], bf, tag="vt")
                    nc.scalar.activation(vt[:], uT[:],
                                         mybir.ActivationFunctionType.Relu)
                    t2_ps = pt2.tile([128, OUTC], f32, tag="pt2")
                    for k in range(2):
                        nc.tensor.matmul(t2_ps[:], lhsT=vt[:, k, :],
                                         rhs=w2_s[:, k, :],
                                         start=(k == 0), stop=(k == 1))
                    nc.vector.tensor_copy(own2_s[:, w, :], t2_ps[:])
                    if w < WA:
                        nc.sync.dma_start(ag2a_in[w * 128 : (w + 1) * 128, :],
                                          own2_s[:, w, :])
                    else:
                        nc.sync.dma_start(ag2b_in[(w - WA) * 128 : (w - WA + 1) * 128, :],
                                          own2_s[:, w, :])

            with nc.named_scope("p3_l1b"):
                for gi in GORDER[: NGRP - NGA]:
                    l1_group(gi)
            # AG2 for sub-table B launches while L1 still works on A-side groups
            with nc.named_scope("ag2b"):
                nc.gpsimd.collective_compute(
                    "AllGather", mybir.AluOpType.bypass,
                    replica_groups=[list(range(NCORES))],
                    ins=[ag2b_in.opt()], outs=[tb2b.opt()])
            with nc.named_scope("p3_l1a"):
                for gi in GORDER[NGRP - NGA :]:
                    l1_group(gi)
            with nc.named_scope("ag2a"):
                nc.gpsimd.collective_compute(
                    "AllGather", mybir.AluOpType.bypass,
                    replica_groups=[list(range(NCORES))],
                    ins=[ag2a_in.opt()], outs=[tb2a.opt()])

            # ---- layer-2 aggregation, two stages so AG latency hides
            partials = {}
            with nc.named_scope("p6_b"):
                # stage 1: self + sub-B messages -> partial (frees PSUM quickly)
                for gi in range(NGRP):
                    ws = list(range(gi * GRP, min((gi + 1) * GRP, WPC)))
                    m_s, S_s = gather_unit(gi, 1, tb2b[:, :], OUTC, qctr)
                    for w in ws:
                        ps = pp1.tile([128, OUTC], f32, tag="ps1")
                        started = False
                        if m_s is not None and Twh[w, 1] > 0:
                            win_mms(w, 1, ps, m_s, S_s, True, False)
                            started = True
                        nc.tensor.matmul(ps[:], lhsT=ident_s[:], rhs=own2_s[:, w, :],
                                         start=not started, stop=True)
                        pp = ppool.tile([128, OUTC], bf, tag="partial")
                        nc.vector.tensor_copy(pp[:], ps[:])
                        partials[w] = pp
            with nc.named_scope("p6_a"):
                # stage 2: sub-A messages + partial -> output
                for gi in range(NGRP):
                    ws = list(range(gi * GRP, min((gi + 1) * GRP, WPC)))
                    m_s, S_s = gather_unit(gi, 0, tb2a[:, :], OUTC, qctr)
                    for w in ws:
                        o_s = epool.tile([128, OUTC], f32, tag="o")
                        if m_s is not None and Twh[w, 0] > 0:
                            ps = pp1.tile([128, OUTC], f32, tag="ps1")
                            win_mms(w, 0, ps, m_s, S_s, True, True)
                            acc = epool.tile([128, OUTC], f32, tag="acc")
                            nc.vector.tensor_add(acc[:], ps[:], partials[w][:])
                        else:
                            acc = partials[w]
                        nc.scalar.activation(o_s[:], acc[:],
                                             mybir.ActivationFunctionType.Copy,
                                             scale=dc1_s[:, w : w + 1])
                        nc.sync.dma_start(out_d[w * 128 : (w + 1) * 128, :], o_s[:])

    nc.compile()
    return nc


def kernel(x, edge_index, W1, b1, W2, b2):
    x = np.asarray(x, np.float32)
    W1 = np.asarray(W1, np.float32)
    W2 = np.asarray(W2, np.float32)
    assert not np.any(np.asarray(b1)) and not np.any(np.asarray(b2)), \
        "kernel assumes zero biases (as in the reference setup)"

    idx16, slots, Twh, base, TT, dcol1, dcol2, dinv = _preprocess(np.asarray(edge_index))
    nc = _build(TT, Twh, base)

    iota = np.broadcast_to(np.arange(128, dtype=np.float32), (128, 128)).astype(ml_dtypes.bfloat16)
    ident = np.eye(128, dtype=np.float32).astype(ml_dtypes.bfloat16)
    # W1 as [ch_k, o_j] 128x128 blocks (lhsT operands for the uT matmuls)
    w1b_in = np.ascontiguousarray(
        W1.reshape(2, 128, 2, 128).transpose(0, 2, 1, 3)).astype(ml_dtypes.bfloat16)
    w2_in = np.ascontiguousarray(W2.reshape(2, 128, OUTC)).astype(ml_dtypes.bfloat16)
    xg = _xg_table(x, dinv)

    xd = (x * dinv[:, None]).astype(np.float32)
    in_maps = []
    for c in range(NCORES):
        xo = np.zeros((RPAD, INC), np.float32)
        xo[:RPC] = xd[c * RPC : (c + 1) * RPC]
        xo = np.ascontiguousarray(
            xo.reshape(WPC, 128, INC).transpose(1, 0, 2)).astype(ml_dtypes.bfloat16)
        in_maps.append({
            "xg": xg, "xo": xo,
            "w1b": w1b_in, "w2": w2_in, "iota": iota, "ident": ident,
            "dcol1": dcol1[c], "dcol2": dcol2[c],
            "idx": idx16[c], "slots": slots[c],
        })

    trace = bool(int(os.environ.get("GCN_KERNEL_TRACE", "0")))
    try:
        res = run_bass_kernel_spmd(nc, in_maps, core_ids=list(range(NCORES)), trace=trace)
    except Exception:
        # rare transient NRT exec failure: retry once on a fresh dispatch
        time_mod = __import__("time"); time_mod.sleep(2.0)
        res = run_bass_kernel_spmd(nc, in_maps, core_ids=list(range(NCORES)), trace=False)
    kernel.last_results = res
    if trace:
        print(f"HW exec time: {res.exec_time_ns} ns")
        kernel.last_exec_time_ns = res.exec_time_ns

    out = np.concatenate([res.results[c]["out"][:RPC] for c in range(NCORES)], axis=0)
    return out.astype(np.float32)


# revision 31
# speedup vs baseline: 1.0946x; 1.0668x over previous
"""GCN encoder (2-layer GCNConv, PyG-style) on 8 Trainium2 NeuronCores.

Sharding: nodes row-sharded 6250/core; edges partitioned by destination-node
owner; per-core segment-sum over 128-dst-slot windows via selection-matrix
matmuls.

Layer 1 is aggregate-then-transform: since the conv is linear before the
nonlinearity, segsum(norm .* (x@W1)[src]) == dinv_d .* segsum(dinv_s .* x[src]) @ W1,
so cores gather dinv.*x rows DIRECTLY from the (host-prepared) input table -
no replicated feature-transform GEMM, no table build on the critical path,
and gathers start at t=0. Per window w the chain is then
    a1  = dinv_d^2 .* segsum1          (scale folded into PSUM evacuation)
    t2  = relu(a1 @ W1) @ W2           (rows of the layer-2 message table)
which is exact for zero biases (as in the reference).

Layer 2: t2 is all-gathered - split into two collectives (sub-tables A/B)
that overlap with remaining gather/compute work - then aggregated the same
way; out = dinv_d .* (segsum2 + t2_own).

Self-loop messages never go through the gather path: their contribution to a
window's segment-sum is one identity matmul from an SBUF-resident copy of the
core's own rows.

Sub-tables (for int16 gather indices and collective splitting): local row
l < 3200 (windows 0-24) -> sub A (8*3200 = 25600 rows); l >= 3200
(windows 25-48) -> sub B (8*3072 = 24576 rows). Both < 2**15.
"""

import os
import numpy as np
import ml_dtypes

import concourse.bacc as bacc
import concourse.tile as tile
from concourse import bass, mybir
from concourse.bass_utils import run_bass_kernel_spmd
from concourse.library_config import mlp

N = 50000
INC, HID, OUTC = 256, 256, 128
NCORES = 8
RPC = N // NCORES            # 6250 rows per core
WPC = (RPC + 127) // 128     # 49 windows per core
RPAD = WPC * 128             # 6272
LSPL = 3200                  # sub-table split on local row (windows 0..24 | 25..48)
NA = NCORES * LSPL           # 25600 rows in sub-table A
NB = NCORES * (RPAD - LSPL)  # 24576 rows in sub-table B
WA = LSPL // 128             # 25 windows in A
GRP = 1                      # windows per supergather group
NGRP = (WPC + GRP - 1) // GRP
# L1 processes B-side groups first so AG2(B) can launch early.
# group NGA = WA//GRP straddles the A/B boundary (windows 24,25) and is
# processed in the B phase, so after the B phase windows 24..48 are all done.
NGA = WA // GRP
GORDER = list(range(NGA, NGRP)) + list(range(0, NGA))


def _preprocess(edge_index):
    """Edge partitioning / ordering and normalization constants (host, index-only)."""
    src = np.asarray(edge_index[0], np.int64)
    dst = np.asarray(edge_index[1], np.int64)

    # degrees include the self-loops the reference adds
    deg = (np.bincount(dst, minlength=N) + 1).astype(np.float64)
    dinv = (1.0 / np.sqrt(deg)).astype(np.float32)

    owner = dst // RPC
    dstl = dst - owner * RPC
    win = dstl >> 7
    slot = dstl & 127
    srho = src // RPC
    srl = src - srho * RPC
    sub = (srl >= LSPL).astype(np.int64)
    gl = np.where(sub == 0, srho * LSPL + srl,
                  srho * (RPAD - LSPL) + (srl - LSPL)).astype(np.int32)

    key = (owner * WPC + win) * 2 + sub
    order = np.argsort(key, kind="stable")
    key_s = key[order]
    gl_s = gl[order]
    slot_s = slot[order].astype(np.int32)

    nbuck = NCORES * WPC * 2
    counts = np.bincount(key_s, minlength=nbuck).reshape(NCORES, WPC, 2)
    starts_flat = np.concatenate([[0], np.cumsum(counts.reshape(-1))])

    # tiles per (window, sub): max over cores so one SPMD program fits all
    Twh = (counts.max(axis=0) + 127) // 128     # [WPC, 2]
    TT = int(Twh.sum())
    # stream order: group -> sub -> window in group -> tiles
    base = np.zeros((WPC, 2), np.int64)
    pos = 0
    for gi in range(NGRP):
        ws = range(gi * GRP, min((gi + 1) * GRP, WPC))
        for h in range(2):
            for w in ws:
                base[w, h] = pos
                pos += Twh[w, h]
    assert pos == TT

    idx_seq = np.zeros((NCORES, TT * 128), np.int32)
    slot_seq = np.full((NCORES, TT * 128), 128, np.int32)  # 128 = dropped sentinel
    for c in range(NCORES):
        for w in range(WPC):
            for h in range(2):
                n = counts[c, w, h]
                if n == 0:
                    continue
                s0 = starts_flat[(c * WPC + w) * 2 + h]
                p0 = base[w, h] * 128
                idx_seq[c, p0 : p0 + n] = gl_s[s0 : s0 + n]
                slot_seq[c, p0 : p0 + n] = slot_s[s0 : s0 + n]

    # wrapped int16 gather-index layout: element j at [j%16, j//16], replicated x8
    idx16 = np.empty((NCORES, 128, TT * 8), np.int16)
    slots = np.empty((NCORES, 128, TT), ml_dtypes.bfloat16)
    for c in range(NCORES):
        a = idx_seq[c].astype(np.int16).reshape(-1, 16).T
        idx16[c] = np.tile(a, (8, 1))
        slots[c] = slot_seq[c].astype(ml_dtypes.bfloat16).reshape(TT, 128).T

    # per-core per-window dinv columns for own rows
    dcol1 = np.zeros((NCORES, 128, WPC), np.float32)
    for c in range(NCORES):
        d = np.zeros(RPAD, np.float32)
        d[:RPC] = dinv[c * RPC : (c + 1) * RPC]
        dcol1[c] = d.reshape(WPC, 128).T
    dcol2 = dcol1 * dcol1

    # per-core per-(window,sub) gather counts, padded to 16: the runtime
    # num_idxs_reg trims each unit's trailing padding descriptors (~10%)
    cnt16 = np.minimum((counts + 15) // 16 * 16,
                       (Twh * 128)[None]).astype(np.int32).reshape(NCORES, WPC * 2)

    return idx16, slots, Twh, base, TT, dcol1, dcol2, dinv, cnt16


def _xg_table(x, dinv):
    """dinv .* x rows in [A | B] rank-major padded order, bf16 (the L1 gather table)."""
    xd = (x * dinv[:, None]).astype(np.float32)
    xg = np.zeros((NA + NB, INC), np.float32)
    nb = RPAD - LSPL
    for rho in range(NCORES):
        xs = xd[rho * RPC : (rho + 1) * RPC]         # [6250, 256]
        xg[rho * LSPL : (rho + 1) * LSPL] = xs[:LSPL]
        xg[NA + rho * nb : NA + rho * nb + (RPC - LSPL)] = xs[LSPL:]
    return np.ascontiguousarray(xg).astype(ml_dtypes.bfloat16)


def _build(TT, Twh, base):
    # 32KB/partition descriptor scratch: the default 16KB holds only ~3.5
    # gathers' descriptors, so a 4th in-flight gather stalls the Pool engine
    # waiting for ring space.
    nc = bacc.Bacc("TRN2", num_devices=NCORES, num_swdge_queues=4,
                   dynamic_dma_scratch_size=32768)
    f32 = mybir.dt.float32
    bf = mybir.dt.bfloat16

    xg_d = nc.dram_tensor("xg", [NA + NB, INC], bf, kind="ExternalInput")
    xo_d = nc.dram_tensor("xo", [128, WPC, INC], bf, kind="ExternalInput")
    w1b_d = nc.dram_tensor("w1b", [2, 2, 128, 128], bf, kind="ExternalInput")
    w2_d = nc.dram_tensor("w2", [2, 128, OUTC], bf, kind="ExternalInput")
    iota_d = nc.dram_tensor("iota", [128, 128], bf, kind="ExternalInput")
    ident_d = nc.dram_tensor("ident", [128, 128], bf, kind="ExternalInput")
    dc1_d = nc.dram_tensor("dcol1", [128, WPC], f32, kind="ExternalInput")
    dc2_d = nc.dram_tensor("dcol2", [128, WPC], f32, kind="ExternalInput")
    idx_d = nc.dram_tensor("idx", [128, TT * 8], mybir.dt.int16, kind="ExternalInput")
    slots_d = nc.dram_tensor("slots", [128, TT], bf, kind="ExternalInput")
    cnt_d = nc.dram_tensor("cnts", [1, WPC * 2], mybir.dt.int32, kind="ExternalInput")
    out_d = nc.dram_tensor("out", [RPAD, OUTC], f32, kind="ExternalOutput")

    # tiles per supergather (group, sub)
    Tg = np.zeros((NGRP, 2), np.int64)
    for gi in range(NGRP):
        ws = range(gi * GRP, min((gi + 1) * GRP, WPC))
        for h in range(2):
            Tg[gi, h] = sum(int(Twh[w, h]) for w in ws)

    with tile.TileContext(nc) as tc:
        nc.gpsimd.load_library(mlp)
        with (
            tc.tile_pool(name="const", bufs=1) as cpool,
            tc.tile_pool(name="gt", bufs=1) as gtpool,
            tc.tile_pool(name="evac", bufs=4) as epool,
            tc.tile_pool(name="tsp", bufs=6) as tpool,
            tc.tile_pool(name="msg", bufs=12) as mpool,
            tc.tile_pool(name="sel", bufs=8) as spool,
            tc.tile_pool(name="part", bufs=WPC) as ppool,
            # PSUM is 8 banks; every buffer costs a full bank
            tc.tile_pool(name="ps1", bufs=3, space="PSUM") as pp1,
            tc.tile_pool(name="pu", bufs=2, space="PSUM") as pu,
            tc.tile_pool(name="pt2", bufs=1, space="PSUM") as pt2,
            tc.tile_pool(name="ptr", bufs=2, space="PSUM") as ptr,
            tc.tile_pool(name="dram", bufs=1, space="DRAM") as dram,
        ):
            # ---- constants to SBUF
            w1b_s = cpool.tile([128, 2, 2, 128], bf)    # W1 as [ch_k][o_j] blocks
            w2_s = cpool.tile([128, 2, OUTC], bf)
            iota_s = cpool.tile([128, 128], bf)
            ident_s = cpool.tile([128, 128], bf)
            dc1_s = cpool.tile([128, WPC], f32)
            dc2_s = cpool.tile([128, WPC], f32)
            idx_s = cpool.tile([128, TT * 8], mybir.dt.int16)
            slots_s = cpool.tile([128, TT], bf)
            cnt_s = cpool.tile([1, WPC * 2], mybir.dt.int32)
            xo_s = gtpool.tile([128, WPC, INC], bf)     # own dinv.*x rows
            own2_s = gtpool.tile([128, WPC, OUTC], bf)  # own table2 rows
            nc.sync.dma_start(idx_s[:], idx_d[:])
            nc.sync.dma_start(slots_s[:], slots_d[:])
            nc.sync.dma_start(cnt_s[:], cnt_d[:])
            for k in range(2):
                for j in range(2):
                    nc.sync.dma_start(w1b_s[:, k, j, :], w1b_d[k, j])
                nc.sync.dma_start(w2_s[:, k, :], w2_d[k])
            nc.sync.dma_start(iota_s[:], iota_d[:])
            nc.sync.dma_start(ident_s[:], ident_d[:])
            nc.sync.dma_start(dc1_s[:], dc1_d[:])
            nc.sync.dma_start(dc2_s[:], dc2_d[:])
            nc.scalar.dma_start(xo_s[:], xo_d[:])

            # zero the msg ring buffers once so padding-skipped rows are finite
            TMAX = int(Tg.max())
            for _ in range(12):
                mz = mpool.tile([128, TMAX, INC], bf, tag="msg")
                nc.vector.memset(mz[:], 0)

            ag2a_in = dram.tile([LSPL, OUTC], bf)
            ag2b_in = dram.tile([RPAD - LSPL, OUTC], bf)
            tb2a = dram.tile([NA, OUTC], bf)
            tb2b = dram.tile([NB, OUTC], bf)

            # ---- edge aggregation unit: gather + S build for one (group, sub)
            def gather_unit(gi, h, tbl_ap, width, qctr):
                T = int(Tg[gi, h])
                if T == 0:
                    return None, None
                ws = list(range(gi * GRP, min((gi + 1) * GRP, WPC)))
                b = int(base[ws[0], h])
                m_s = mpool.tile([128, T, width], bf, tag="msg")
                nc.gpsimd.dma_gather(
                    m_s[:], tbl_ap, idx_s[:, b * 8 : (b + T) * 8],
                    T * 128, T * 128, width,
                    single_packet=False, queue_num=qctr[0] % 4)
                qctr[0] += 1
                S_s = spool.tile([128, T, 128], bf, tag="sel")
                nc.vector.tensor_tensor(
                    out=S_s[:],
                    in0=slots_s[:, b : b + T, None].to_broadcast([128, T, 128]),
                    in1=iota_s[:, None, :].to_broadcast([128, T, 128]),
                    op=mybir.AluOpType.is_equal)
                return m_s, S_s

            def win_mms(w, h, ps, m_s, S_s, first, last):
                # base of this unit's stream is base[first window of group, h]
                gw0 = (w // GRP) * GRP
                b = int(base[gw0, h])
                n = int(Twh[w, h])
                for t in range(n):
                    tt = int(base[w, h]) - b + t
                    nc.tensor.matmul(ps[:], lhsT=S_s[:, tt, :], rhs=m_s[:, tt, :],
                                     start=(first and t == 0),
                                     stop=(last and t == n - 1))

            qctr = [0]

            # ---- layer-1 aggregation + per-window transform (B-side groups first)
            def l1_group(gi):
                ws = list(range(gi * GRP, min((gi + 1) * GRP, WPC)))
                units = {}
                for h in range(2):
                    units[h] = gather_unit(
                        gi, h, xg_d[:NA, :] if h == 0 else xg_d[NA:, :], INC, qctr)
                pss = {}
                for w in ws:
                    ps = pp1.tile([128, HID], f32, tag="ps1")
                    pss[w] = ps
                    last_h = max((h for h in range(2)
                                  if units[h][0] is not None and Twh[w, h] > 0),
                                 default=None)
                    # self-loop contribution first (operands ready at t=0)
                    nc.tensor.matmul(ps[:], lhsT=ident_s[:], rhs=xo_s[:, w, :],
                                     start=True, stop=last_h is None)
                    for h in range(2):
                        m_s, S_s = units[h]
                        if m_s is None or Twh[w, h] == 0:
                            continue
                        win_mms(w, h, ps, m_s, S_s, False, h == last_h)
                for w in ws:
                    ps = pss[w]
                    # a1 = dinv^2 .* segsum1
                    a1 = epool.tile([128, HID], bf, tag="a1")
                    nc.scalar.activation(a1[:], ps[:],
                                         mybir.ActivationFunctionType.Copy,
                                         scale=dc2_s[:, w : w + 1])
                    # a1^T via 2 PE transposes
                    a1t = tpool.tile([128, 2, 128], bf, tag="a1t")
                    for k in range(2):
                        pt = ptr.tile([128, 128], bf, tag="pt")
                        nc.tensor.transpose(pt[:], a1[:, k * 128 : (k + 1) * 128],
                                            ident_s[:])
                        nc.vector.tensor_copy(a1t[:, k, :], pt[:])
                    # u^T = (a1 @ W1)^T directly: lhsT = W1 block, rhs = a1^T
                    uT = pu.tile([128, 2, 128], f32, tag="uT")
                    for j in range(2):
                        for k in range(2):
                            nc.tensor.matmul(uT[:, j, :], lhsT=w1b_s[:, k, j, :],
                                             rhs=a1t[:, k, :],
                                             start=(k == 0), stop=(k == 1))
                    # v^T = relu(u^T) is directly the lhsT for W2
                    vt = tpool.tile([128, 2, 128], bf, tag="vt")
                    nc.scalar.activation(vt[:], uT[:],
                                         mybir.ActivationFunctionType.Relu)
                    t2_ps = pt2.tile([128, OUTC], f32, tag="pt2")
                    for k in range(2):
                        nc.tensor.matmul(t2_ps[:], lhsT=vt[:, k, :],
                                         rhs=w2_s[:, k, :],
                                         start=(k == 0), stop=(k == 1))
                    nc.vector.tensor_copy(own2_s[:, w, :], t2_ps[:])
                    if w < WA:
                        nc.sync.dma_start(ag2a_in[w * 128 : (w + 1) * 128, :],
                                          own2_s[:, w, :])
                    else:
                        nc.sync.dma_start(ag2b_in[(w - WA) * 128 : (w - WA + 1) * 128, :],
                                          own2_s[:, w, :])

            with nc.named_scope("p3_l1b"):
                for gi in GORDER[: NGRP - NGA]:
                    l1_group(gi)
            # AG2(B) launches a few groups into the A loop: the launch
            # instruction waits for the B t2-row DMAs, and the gathers queued
            # ahead of it keep the SWDGE queues draining meanwhile.
            with nc.named_scope("p3_l1a"):
                for j, gi in enumerate(GORDER[NGRP - NGA :]):
                    l1_group(gi)
                    if j == 2:
                        with nc.named_scope("ag2b"):
                            nc.gpsimd.collective_compute(
                                "AllGather", mybir.AluOpType.bypass,
                                replica_groups=[list(range(NCORES))],
                                ins=[ag2b_in.opt()], outs=[tb2b.opt()])

            # ---- layer-2 aggregation, two stages so AG latency hides
            partials = {}
            with nc.named_scope("p6_b"):
                # stage 1: self + sub-B messages -> partial (frees PSUM quickly)
                for gi in range(NGRP):
                    if gi == 5:
                        with nc.named_scope("ag2a"):
                            nc.gpsimd.collective_compute(
                                "AllGather", mybir.AluOpType.bypass,
                                replica_groups=[list(range(NCORES))],
                                ins=[ag2a_in.opt()], outs=[tb2a.opt()])
                    ws = list(range(gi * GRP, min((gi + 1) * GRP, WPC)))
                    m_s, S_s = gather_unit(gi, 1, tb2b[:, :], OUTC, qctr)
                    for w in ws:
                        ps = pp1.tile([128, OUTC], f32, tag="ps1")
                        started = False
                        if m_s is not None and Twh[w, 1] > 0:
                            win_mms(w, 1, ps, m_s, S_s, True, False)
                            started = True
                        nc.tensor.matmul(ps[:], lhsT=ident_s[:], rhs=own2_s[:, w, :],
                                         start=not started, stop=True)
                        pp = ppool.tile([128, OUTC], bf, tag="partial")
                        nc.vector.tensor_copy(pp[:], ps[:])
                        partials[w] = pp
            with nc.named_scope("p6_a"):
                # stage 2: sub-A messages + partial -> output
                for gi in range(NGRP):
                    ws = list(range(gi * GRP, min((gi + 1) * GRP, WPC)))
                    m_s, S_s = gather_unit(gi, 0, tb2a[:, :], OUTC, qctr)
                    for w in ws:
                        o_s = epool.tile([128, OUTC], f32, tag="o")
                        if m_s is not None and Twh[w, 0] > 0:
                            ps = pp1.tile([128, OUTC], f32, tag="ps1")
                            win_mms(w, 0, ps, m_s, S_s, True, True)
                            acc = epool.tile([128, OUTC], f32, tag="acc")
                            nc.vector.tensor_add(acc[:], ps[:], partials[w][:])
                        else:
                            acc = partials[w]
                        nc.scalar.activation(o_s[:], acc[:],
                                             mybir.ActivationFunctionType.Copy,
                                             scale=dc1_s[:, w : w + 1])
                        nc.sync.dma_start(out_d[w * 128 : (w + 1) * 128, :], o_s[:])

    nc.compile()
    return nc


def kernel(x, edge_index, W1, b1, W2, b2):
    x = np.asarray(x, np.float32)
    W1 = np.asarray(W1, np.float32)
    W2 = np.asarray(W2, np.float32)
    assert not np.any(np.asarray(b1)) and not np.any(np.asarray(b2)), \
        "kernel assumes zero biases (as in the reference setup)"

    idx16, slots, Twh, base, TT, dcol1, dcol2, dinv, cnt16 = _preprocess(np.asarray(edge_index))
    nc = _build(TT, Twh, base)

    iota = np.broadcast_to(np.arange(128, dtype=np.float32), (128, 128)).astype(ml_dtypes.bfloat16)
    ident = np.eye(128, dtype=np.float32).astype(ml_dtypes.bfloat16)
    # W1 as [ch_k, o_j] 128x128 blocks (lhsT operands for the uT matmuls)
    w1b_in = np.ascontiguousarray(
        W1.reshape(2, 128, 2, 128).transpose(0, 2, 1, 3)).astype(ml_dtypes.bfloat16)
    w2_in = np.ascontiguousarray(W2.reshape(2, 128, OUTC)).astype(ml_dtypes.bfloat16)
    xg = _xg_table(x, dinv)

    xd = (x * dinv[:, None]).astype(np.float32)
    in_maps = []
    for c in range(NCORES):
        xo = np.zeros((RPAD, INC), np.float32)
        xo[:RPC] = xd[c * RPC : (c + 1) * RPC]
        xo = np.ascontiguousarray(
            xo.reshape(WPC, 128, INC).transpose(1, 0, 2)).astype(ml_dtypes.bfloat16)
        in_maps.append({
            "xg": xg, "xo": xo,
            "w1b": w1b_in, "w2": w2_in, "iota": iota, "ident": ident,
            "dcol1": dcol1[c], "dcol2": dcol2[c],
            "idx": idx16[c], "slots": slots[c],
            "cnts": cnt16[c][None, :],
        })

    trace = bool(int(os.environ.get("GCN_KERNEL_TRACE", "0")))
    try:
        res = run_bass_kernel_spmd(nc, in_maps, core_ids=list(range(NCORES)), trace=trace)
    except Exception:
        # rare transient NRT exec failure: retry once on a fresh dispatch
        time_mod = __import__("time"); time_mod.sleep(2.0)
        res = run_bass_kernel_spmd(nc, in_maps, core_ids=list(range(NCORES)), trace=False)
    kernel.last_results = res
    if trace:
        print(f"HW exec time: {res.exec_time_ns} ns")
        kernel.last_exec_time_ns = res.exec_time_ns

    out = np.concatenate([res.results[c]["out"][:RPC] for c in range(NCORES)], axis=0)
    return out.astype(np.float32)


# revision 35
# speedup vs baseline: 1.3656x; 1.2476x over previous
"""GCN encoder (2-layer GCNConv, PyG-style) on 8 Trainium2 NeuronCores.

Sharding: nodes row-sharded 6250/core; edges partitioned by destination-node
owner; per-core segment-sum over 128-dst-slot windows via selection-matrix
matmuls.

Layer 1 is aggregate-then-transform: since the conv is linear before the
nonlinearity, segsum(norm .* (x@W1)[src]) == dinv_d .* segsum(dinv_s .* x[src]) @ W1,
so cores gather dinv.*x rows DIRECTLY from the (host-prepared) input table -
no replicated feature-transform GEMM, no table build on the critical path,
and gathers start at t=0. Per window w the chain is then
    a1  = dinv_d^2 .* segsum1          (scale folded into PSUM evacuation)
    t2  = relu(a1 @ W1) @ W2           (rows of the layer-2 message table)
which is exact for zero biases (as in the reference).

Layer 2: t2 is all-gathered - split into two collectives (sub-tables A/B)
that overlap with remaining gather/compute work - then aggregated the same
way; out = dinv_d .* (segsum2 + t2_own).

Self-loop messages never go through the gather path: their contribution to a
window's segment-sum is one identity matmul from an SBUF-resident copy of the
core's own rows.

Sub-tables (for int16 gather indices and collective splitting): local row
l < 3200 (windows 0-24) -> sub A (8*3200 = 25600 rows); l >= 3200
(windows 25-48) -> sub B (8*3072 = 24576 rows). Both < 2**15.
"""

import os
import numpy as np
import ml_dtypes

import concourse.bacc as bacc
import concourse.tile as tile
from concourse import bass, mybir
from concourse.bass_utils import run_bass_kernel_spmd
from concourse.library_config import mlp

N = 50000
INC, HID, OUTC = 256, 256, 128
NCORES = 8
RPC = N // NCORES            # 6250 rows per core
WPC = (RPC + 127) // 128     # 49 windows per core
RPAD = WPC * 128             # 6272
LSPL = 3200                  # sub-table split on local row (windows 0..24 | 25..48)
NA = NCORES * LSPL           # 25600 rows in sub-table A
NB = NCORES * (RPAD - LSPL)  # 24576 rows in sub-table B
WA = LSPL // 128             # 25 windows in A
GRP = 1                      # windows per supergather group
NGRP = (WPC + GRP - 1) // GRP
# L1 processes B-side groups first so AG2(B) can launch early.
# group NGA = WA//GRP straddles the A/B boundary (windows 24,25) and is
# processed in the B phase, so after the B phase windows 24..48 are all done.
NGA = WA // GRP
GORDER = list(range(NGA, NGRP)) + list(range(0, NGA))


def _preprocess(edge_index):
    """Edge partitioning / ordering and normalization constants (host, index-only)."""
    src = np.asarray(edge_index[0], np.int64)
    dst = np.asarray(edge_index[1], np.int64)

    # degrees include the self-loops the reference adds
    deg = (np.bincount(dst, minlength=N) + 1).astype(np.float64)
    dinv = (1.0 / np.sqrt(deg)).astype(np.float32)

    owner = dst // RPC
    dstl = dst - owner * RPC
    win = dstl >> 7
    slot = dstl & 127
    srho = src // RPC
    srl = src - srho * RPC
    sub = (srl >= LSPL).astype(np.int64)
    gl = np.where(sub == 0, srho * LSPL + srl,
                  srho * (RPAD - LSPL) + (srl - LSPL)).astype(np.int32)

    key = (owner * WPC + win) * 2 + sub
    order = np.argsort(key, kind="stable")
    key_s = key[order]
    gl_s = gl[order]
    slot_s = slot[order].astype(np.int32)

    nbuck = NCORES * WPC * 2
    counts = np.bincount(key_s, minlength=nbuck).reshape(NCORES, WPC, 2)
    starts_flat = np.concatenate([[0], np.cumsum(counts.reshape(-1))])

    # tiles per (window, sub): max over cores so one SPMD program fits all
    Twh = (counts.max(axis=0) + 127) // 128     # [WPC, 2]
    TT = int(Twh.sum())
    # stream order: group -> sub -> window in group -> tiles
    base = np.zeros((WPC, 2), np.int64)
    pos = 0
    for gi in range(NGRP):
        ws = range(gi * GRP, min((gi + 1) * GRP, WPC))
        for h in range(2):
            for w in ws:
                base[w, h] = pos
                pos += Twh[w, h]
    assert pos == TT

    idx_seq = np.zeros((NCORES, TT * 128), np.int32)
    slot_seq = np.full((NCORES, TT * 128), 128, np.int32)  # 128 = dropped sentinel
    for c in range(NCORES):
        for w in range(WPC):
            for h in range(2):
                n = counts[c, w, h]
                if n == 0:
                    continue
                s0 = starts_flat[(c * WPC + w) * 2 + h]
                p0 = base[w, h] * 128
                idx_seq[c, p0 : p0 + n] = gl_s[s0 : s0 + n]
                slot_seq[c, p0 : p0 + n] = slot_s[s0 : s0 + n]

    # wrapped int16 gather-index layout: element j at [j%16, j//16], replicated x8
    idx16 = np.empty((NCORES, 128, TT * 8), np.int16)
    slots = np.empty((NCORES, 128, TT), ml_dtypes.bfloat16)
    for c in range(NCORES):
        a = idx_seq[c].astype(np.int16).reshape(-1, 16).T
        idx16[c] = np.tile(a, (8, 1))
        slots[c] = slot_seq[c].astype(ml_dtypes.bfloat16).reshape(TT, 128).T

    # per-core per-window dinv columns for own rows
    dcol1 = np.zeros((NCORES, 128, WPC), np.float32)
    for c in range(NCORES):
        d = np.zeros(RPAD, np.float32)
        d[:RPC] = dinv[c * RPC : (c + 1) * RPC]
        dcol1[c] = d.reshape(WPC, 128).T
    dcol2 = dcol1 * dcol1

    # static per-(window,sub) gather length: the max edge count over cores.
    # Trailing tile padding beyond it is never fetched; those message rows
    # keep stale-but-finite data that the sentinel S rows zero out.
    cmax = counts.max(axis=0).astype(np.int64)   # [WPC, 2]

    return idx16, slots, Twh, base, TT, dcol1, dcol2, dinv, cmax


def _xg_table(x, dinv):
    """dinv .* x rows in [A | B] rank-major padded order, bf16 (the L1 gather table)."""
    xd = (x * dinv[:, None]).astype(np.float32)
    xg = np.zeros((NA + NB, INC), np.float32)
    nb = RPAD - LSPL
    for rho in range(NCORES):
        xs = xd[rho * RPC : (rho + 1) * RPC]         # [6250, 256]
        xg[rho * LSPL : (rho + 1) * LSPL] = xs[:LSPL]
        xg[NA + rho * nb : NA + rho * nb + (RPC - LSPL)] = xs[LSPL:]
    return np.ascontiguousarray(xg).astype(ml_dtypes.bfloat16)


def _build(TT, Twh, base, cmax):
    nc = bacc.Bacc("TRN2", num_devices=NCORES, num_swdge_queues=4)
    f32 = mybir.dt.float32
    bf = mybir.dt.bfloat16

    xg_d = nc.dram_tensor("xg", [NA + NB, INC], bf, kind="ExternalInput")
    xo_d = nc.dram_tensor("xo", [128, WPC, INC], bf, kind="ExternalInput")
    w1b_d = nc.dram_tensor("w1b", [2, 2, 128, 128], bf, kind="ExternalInput")
    w2_d = nc.dram_tensor("w2", [2, 128, OUTC], bf, kind="ExternalInput")
    iota_d = nc.dram_tensor("iota", [128, 128], bf, kind="ExternalInput")
    ident_d = nc.dram_tensor("ident", [128, 128], bf, kind="ExternalInput")
    dc1_d = nc.dram_tensor("dcol1", [128, WPC], f32, kind="ExternalInput")
    dc2_d = nc.dram_tensor("dcol2", [128, WPC], f32, kind="ExternalInput")
    idx_d = nc.dram_tensor("idx", [128, TT * 8], mybir.dt.int16, kind="ExternalInput")
    slots_d = nc.dram_tensor("slots", [128, TT], bf, kind="ExternalInput")
    out_d = nc.dram_tensor("out", [RPAD, OUTC], f32, kind="ExternalOutput")

    # tiles per supergather (group, sub)
    Tg = np.zeros((NGRP, 2), np.int64)
    for gi in range(NGRP):
        ws = range(gi * GRP, min((gi + 1) * GRP, WPC))
        for h in range(2):
            Tg[gi, h] = sum(int(Twh[w, h]) for w in ws)

    with tile.TileContext(nc) as tc:
        nc.gpsimd.load_library(mlp)
        with (
            tc.tile_pool(name="const", bufs=1) as cpool,
            tc.tile_pool(name="gt", bufs=1) as gtpool,
            tc.tile_pool(name="evac", bufs=4) as epool,
            tc.tile_pool(name="tsp", bufs=6) as tpool,
            tc.tile_pool(name="msg", bufs=12) as mpool,
            tc.tile_pool(name="sel", bufs=8) as spool,
            tc.tile_pool(name="part", bufs=WPC) as ppool,
            # PSUM is 8 banks; every buffer costs a full bank
            tc.tile_pool(name="ps1", bufs=3, space="PSUM") as pp1,
            tc.tile_pool(name="pu", bufs=2, space="PSUM") as pu,
            tc.tile_pool(name="pt2", bufs=1, space="PSUM") as pt2,
            tc.tile_pool(name="ptr", bufs=2, space="PSUM") as ptr,
            tc.tile_pool(name="dram", bufs=1, space="DRAM") as dram,
        ):
            # ---- constants to SBUF
            w1b_s = cpool.tile([128, 2, 2, 128], bf)    # W1 as [ch_k][o_j] blocks
            w2_s = cpool.tile([128, 2, OUTC], bf)
            iota_s = cpool.tile([128, 128], bf)
            ident_s = cpool.tile([128, 128], bf)
            dc1_s = cpool.tile([128, WPC], f32)
            dc2_s = cpool.tile([128, WPC], f32)
            idx_s = cpool.tile([128, TT * 8], mybir.dt.int16)
            slots_s = cpool.tile([128, TT], bf)
            xo_s = gtpool.tile([128, WPC, INC], bf)     # own dinv.*x rows
            own2_s = gtpool.tile([128, WPC, OUTC], bf)  # own table2 rows
            nc.sync.dma_start(idx_s[:], idx_d[:])
            nc.sync.dma_start(slots_s[:], slots_d[:])
            for k in range(2):
                for j in range(2):
                    nc.sync.dma_start(w1b_s[:, k, j, :], w1b_d[k, j])
                nc.sync.dma_start(w2_s[:, k, :], w2_d[k])
            nc.sync.dma_start(iota_s[:], iota_d[:])
            nc.sync.dma_start(ident_s[:], ident_d[:])
            nc.sync.dma_start(dc1_s[:], dc1_d[:])
            nc.sync.dma_start(dc2_s[:], dc2_d[:])
            nc.scalar.dma_start(xo_s[:], xo_d[:])

            # zero the msg ring buffers once so padding-skipped rows are finite
            TMAX = int(Tg.max())
            for _ in range(12):
                mz = mpool.tile([128, TMAX, INC], bf, tag="msg")
                nc.vector.memset(mz[:], 0)

            ag2a_in = dram.tile([LSPL, OUTC], bf)
            ag2b_in = dram.tile([RPAD - LSPL, OUTC], bf)
            tb2a = dram.tile([NA, OUTC], bf)
            tb2b = dram.tile([NB, OUTC], bf)

            # ---- edge aggregation unit: gather + S build for one (group, sub)
            def gather_unit(gi, h, tbl_ap, width, qctr):
                T = int(Tg[gi, h])
                if T == 0:
                    return None, None
                ws = list(range(gi * GRP, min((gi + 1) * GRP, WPC)))
                b = int(base[ws[0], h])
                m_s = mpool.tile([128, T, width], bf, tag="msg")
                n_idx = int(cmax[gi, h]) if GRP == 1 else T * 128
                nc.gpsimd.dma_gather(
                    m_s[:], tbl_ap, idx_s[:, b * 8 : (b + T) * 8],
                    n_idx, n_idx, width,
                    single_packet=False, queue_num=qctr[0] % 4)
                qctr[0] += 1
                S_s = spool.tile([128, T, 128], bf, tag="sel")
                nc.vector.tensor_tensor(
                    out=S_s[:],
                    in0=slots_s[:, b : b + T, None].to_broadcast([128, T, 128]),
                    in1=iota_s[:, None, :].to_broadcast([128, T, 128]),
                    op=mybir.AluOpType.is_equal)
                return m_s, S_s

            def win_mms(w, h, ps, m_s, S_s, first, last):
                # base of this unit's stream is base[first window of group, h]
                gw0 = (w // GRP) * GRP
                b = int(base[gw0, h])
                n = int(Twh[w, h])
                for t in range(n):
                    tt = int(base[w, h]) - b + t
                    nc.tensor.matmul(ps[:], lhsT=S_s[:, tt, :], rhs=m_s[:, tt, :],
                                     start=(first and t == 0),
                                     stop=(last and t == n - 1))

            qctr = [0]

            # ---- layer-1 aggregation + per-window transform (B-side groups first)
            def l1_group(gi):
                ws = list(range(gi * GRP, min((gi + 1) * GRP, WPC)))
                units = {}
                for h in range(2):
                    units[h] = gather_unit(
                        gi, h, xg_d[:NA, :] if h == 0 else xg_d[NA:, :], INC, qctr)
                pss = {}
                for w in ws:
                    ps = pp1.tile([128, HID], f32, tag="ps1")
                    pss[w] = ps
                    last_h = max((h for h in range(2)
                                  if units[h][0] is not None and Twh[w, h] > 0),
                                 default=None)
                    # self-loop contribution first (operands ready at t=0)
                    nc.tensor.matmul(ps[:], lhsT=ident_s[:], rhs=xo_s[:, w, :],
                                     start=True, stop=last_h is None)
                    for h in range(2):
                        m_s, S_s = units[h]
                        if m_s is None or Twh[w, h] == 0:
                            continue
                        win_mms(w, h, ps, m_s, S_s, False, h == last_h)
                for w in ws:
                    ps = pss[w]
                    # a1 = dinv^2 .* segsum1
                    a1 = epool.tile([128, HID], bf, tag="a1")
                    nc.scalar.activation(a1[:], ps[:],
                                         mybir.ActivationFunctionType.Copy,
                                         scale=dc2_s[:, w : w + 1])
                    # a1^T via 2 PE transposes
                    a1t = tpool.tile([128, 2, 128], bf, tag="a1t")
                    for k in range(2):
                        pt = ptr.tile([128, 128], bf, tag="pt")
                        nc.tensor.transpose(pt[:], a1[:, k * 128 : (k + 1) * 128],
                                            ident_s[:])
                        nc.vector.tensor_copy(a1t[:, k, :], pt[:])
                    # u^T = (a1 @ W1)^T directly: lhsT = W1 block, rhs = a1^T
                    uT = pu.tile([128, 2, 128], f32, tag="uT")
                    for j in range(2):
                        for k in range(2):
                            nc.tensor.matmul(uT[:, j, :], lhsT=w1b_s[:, k, j, :],
                                             rhs=a1t[:, k, :],
                                             start=(k == 0), stop=(k == 1))
                    # v^T = relu(u^T) is directly the lhsT for W2
                    vt = tpool.tile([128, 2, 128], bf, tag="vt")
                    nc.scalar.activation(vt[:], uT[:],
                                         mybir.ActivationFunctionType.Relu)
                    t2_ps = pt2.tile([128, OUTC], f32, tag="pt2")
                    for k in range(2):
                        nc.tensor.matmul(t2_ps[:], lhsT=vt[:, k, :],
                                         rhs=w2_s[:, k, :],
                                         start=(k == 0), stop=(k == 1))
                    nc.vector.tensor_copy(own2_s[:, w, :], t2_ps[:])
                    if w < WA:
                        nc.sync.dma_start(ag2a_in[w * 128 : (w + 1) * 128, :],
                                          own2_s[:, w, :])
                    else:
                        nc.sync.dma_start(ag2b_in[(w - WA) * 128 : (w - WA + 1) * 128, :],
                                          own2_s[:, w, :])

            with nc.named_scope("p3_l1b"):
                for gi in GORDER[: NGRP - NGA]:
                    l1_group(gi)
            # AG2(B) launches a few groups into the A loop: the launch
            # instruction waits for the B t2-row DMAs, and the gathers queued
            # ahead of it keep the SWDGE queues draining meanwhile.
            with nc.named_scope("p3_l1a"):
                for j, gi in enumerate(GORDER[NGRP - NGA :]):
                    l1_group(gi)
                    if j == 2:
                        with nc.named_scope("ag2b"):
                            nc.gpsimd.collective_compute(
                                "AllGather", mybir.AluOpType.bypass,
                                replica_groups=[list(range(NCORES))],
                                ins=[ag2b_in.opt()], outs=[tb2b.opt()])

            # ---- layer-2 aggregation, two stages so AG latency hides
            partials = {}
            with nc.named_scope("p6_b"):
                # stage 1: self + sub-B messages -> partial (frees PSUM quickly)
                for gi in range(NGRP):
                    if gi == 5:
                        with nc.named_scope("ag2a"):
                            nc.gpsimd.collective_compute(
                                "AllGather", mybir.AluOpType.bypass,
                                replica_groups=[list(range(NCORES))],
                                ins=[ag2a_in.opt()], outs=[tb2a.opt()])
                    ws = list(range(gi * GRP, min((gi + 1) * GRP, WPC)))
                    m_s, S_s = gather_unit(gi, 1, tb2b[:, :], OUTC, qctr)
                    for w in ws:
                        ps = pp1.tile([128, OUTC], f32, tag="ps1")
                        started = False
                        if m_s is not None and Twh[w, 1] > 0:
                            win_mms(w, 1, ps, m_s, S_s, True, False)
                            started = True
                        nc.tensor.matmul(ps[:], lhsT=ident_s[:], rhs=own2_s[:, w, :],
                                         start=not started, stop=True)
                        pp = ppool.tile([128, OUTC], bf, tag="partial")
                        nc.vector.tensor_copy(pp[:], ps[:])
                        partials[w] = pp
            with nc.named_scope("p6_a"):
                # stage 2: sub-A messages + partial -> output
                for gi in range(NGRP):
                    ws = list(range(gi * GRP, min((gi + 1) * GRP, WPC)))
                    m_s, S_s = gather_unit(gi, 0, tb2a[:, :], OUTC, qctr)
                    for w in ws:
                        o_s = epool.tile([128, OUTC], f32, tag="o")
                        if m_s is not None and Twh[w, 0] > 0:
                            ps = pp1.tile([128, OUTC], f32, tag="ps1")
                            win_mms(w, 0, ps, m_s, S_s, True, True)
                            acc = epool.tile([128, OUTC], f32, tag="acc")
                            nc.vector.tensor_add(acc[:], ps[:], partials[w][:])
                        else:
                            acc = partials[w]
                        nc.scalar.activation(o_s[:], acc[:],
                                             mybir.ActivationFunctionType.Copy,
                                             scale=dc1_s[:, w : w + 1])
                        nc.sync.dma_start(out_d[w * 128 : (w + 1) * 128, :], o_s[:])

    nc.compile()
    return nc


def kernel(x, edge_index, W1, b1, W2, b2):
    x = np.asarray(x, np.float32)
    W1 = np.asarray(W1, np.float32)
    W2 = np.asarray(W2, np.float32)
    assert not np.any(np.asarray(b1)) and not np.any(np.asarray(b2)), \
        "kernel assumes zero biases (as in the reference setup)"

    idx16, slots, Twh, base, TT, dcol1, dcol2, dinv, cmax = _preprocess(np.asarray(edge_index))
    nc = _build(TT, Twh, base, cmax)

    iota = np.broadcast_to(np.arange(128, dtype=np.float32), (128, 128)).astype(ml_dtypes.bfloat16)
    ident = np.eye(128, dtype=np.float32).astype(ml_dtypes.bfloat16)
    # W1 as [ch_k, o_j] 128x128 blocks (lhsT operands for the uT matmuls)
    w1b_in = np.ascontiguousarray(
        W1.reshape(2, 128, 2, 128).transpose(0, 2, 1, 3)).astype(ml_dtypes.bfloat16)
    w2_in = np.ascontiguousarray(W2.reshape(2, 128, OUTC)).astype(ml_dtypes.bfloat16)
    xg = _xg_table(x, dinv)

    xd = (x * dinv[:, None]).astype(np.float32)
    in_maps = []
    for c in range(NCORES):
        xo = np.zeros((RPAD, INC), np.float32)
        xo[:RPC] = xd[c * RPC : (c + 1) * RPC]
        xo = np.ascontiguousarray(
            xo.reshape(WPC, 128, INC).transpose(1, 0, 2)).astype(ml_dtypes.bfloat16)
        in_maps.append({
            "xg": xg, "xo": xo,
            "w1b": w1b_in, "w2": w2_in, "iota": iota, "ident": ident,
            "dcol1": dcol1[c], "dcol2": dcol2[c],
            "idx": idx16[c], "slots": slots[c],
        })

    trace = bool(int(os.environ.get("GCN_KERNEL_TRACE", "0")))
    try:
        res = run_bass_kernel_spmd(nc, in_maps, core_ids=list(range(NCORES)), trace=trace)
    except Exception:
        # rare transient NRT exec failure: retry once on a fresh dispatch
        time_mod = __import__("time"); time_mod.sleep(2.0)
        res = run_bass_kernel_spmd(nc, in_maps, core_ids=list(range(NCORES)), trace=False)
    kernel.last_results = res
    if trace:
        print(f"HW exec time: {res.exec_time_ns} ns")
        kernel.last_exec_time_ns = res.exec_time_ns

    out = np.concatenate([res.results[c]["out"][:RPC] for c in range(NCORES)], axis=0)
    return out.astype(np.float32)


# revision 37
# speedup vs baseline: 1.4309x; 1.0478x over previous
"""GCN encoder (2-layer GCNConv, PyG-style) on 8 Trainium2 NeuronCores.

Sharding: nodes row-sharded 6250/core; edges partitioned by destination-node
owner; per-core segment-sum over 128-dst-slot windows via selection-matrix
matmuls.

Layer 1 is aggregate-then-transform: since the conv is linear before the
nonlinearity, segsum(norm .* (x@W1)[src]) == dinv_d .* segsum(dinv_s .* x[src]) @ W1,
so cores gather dinv.*x rows DIRECTLY from the (host-prepared) input table -
no replicated feature-transform GEMM, no table build on the critical path,
and gathers start at t=0. Per window w the chain is then
    a1  = dinv_d^2 .* segsum1          (scale folded into PSUM evacuation)
    t2  = relu(a1 @ W1) @ W2           (rows of the layer-2 message table)
which is exact for zero biases (as in the reference).

Layer 2: t2 is all-gathered - split into two collectives (sub-tables A/B)
that overlap with remaining gather/compute work - then aggregated the same
way; out = dinv_d .* (segsum2 + t2_own).

Self-loop messages never go through the gather path: their contribution to a
window's segment-sum is one identity matmul from an SBUF-resident copy of the
core's own rows.

Sub-tables (for int16 gather indices and collective splitting): local row
l < 3200 (windows 0-24) -> sub A (8*3200 = 25600 rows); l >= 3200
(windows 25-48) -> sub B (8*3072 = 24576 rows). Both < 2**15.
"""

import os
import numpy as np
import ml_dtypes

import concourse.bacc as bacc
import concourse.tile as tile
from concourse import bass, mybir
from concourse.bass_utils import run_bass_kernel_spmd
from concourse.library_config import mlp

N = 50000
INC, HID, OUTC = 256, 256, 128
NCORES = 8
RPC = N // NCORES            # 6250 rows per core
WPC = (RPC + 127) // 128     # 49 windows per core
RPAD = WPC * 128             # 6272
LSPL = 3200                  # sub-table split on local row (windows 0..24 | 25..48)
NA = NCORES * LSPL           # 25600 rows in sub-table A
NB = NCORES * (RPAD - LSPL)  # 24576 rows in sub-table B
WA = LSPL // 128             # 25 windows in A
GRP = 1                      # windows per supergather group
NGRP = (WPC + GRP - 1) // GRP
# L1 processes B-side groups first so AG2(B) can launch early.
# group NGA = WA//GRP straddles the A/B boundary (windows 24,25) and is
# processed in the B phase, so after the B phase windows 24..48 are all done.
NGA = WA // GRP
GORDER = list(range(NGA, NGRP)) + list(range(0, NGA))


def _preprocess(edge_index):
    """Edge partitioning / ordering and normalization constants (host, index-only)."""
    src = np.asarray(edge_index[0], np.int64)
    dst = np.asarray(edge_index[1], np.int64)

    # degrees include the self-loops the reference adds
    deg = (np.bincount(dst, minlength=N) + 1).astype(np.float64)
    dinv = (1.0 / np.sqrt(deg)).astype(np.float32)

    owner = dst // RPC
    dstl = dst - owner * RPC
    win = dstl >> 7
    slot = dstl & 127
    srho = src // RPC
    srl = src - srho * RPC
    sub = (srl >= LSPL).astype(np.int64)
    gl = np.where(sub == 0, srho * LSPL + srl,
                  srho * (RPAD - LSPL) + (srl - LSPL)).astype(np.int32)

    key = (owner * WPC + win) * 2 + sub
    order = np.argsort(key, kind="stable")
    key_s = key[order]
    gl_s = gl[order]
    slot_s = slot[order].astype(np.int32)

    nbuck = NCORES * WPC * 2
    counts = np.bincount(key_s, minlength=nbuck).reshape(NCORES, WPC, 2)
    starts_flat = np.concatenate([[0], np.cumsum(counts.reshape(-1))])

    # tiles per (window, sub): max over cores so one SPMD program fits all
    Twh = (counts.max(axis=0) + 127) // 128     # [WPC, 2]
    TT = int(Twh.sum())
    # stream order: group -> sub -> window in group -> tiles
    base = np.zeros((WPC, 2), np.int64)
    pos = 0
    for gi in range(NGRP):
        ws = range(gi * GRP, min((gi + 1) * GRP, WPC))
        for h in range(2):
            for w in ws:
                base[w, h] = pos
                pos += Twh[w, h]
    assert pos == TT

    idx_seq = np.zeros((NCORES, TT * 128), np.int32)
    slot_seq = np.full((NCORES, TT * 128), 128, np.int32)  # 128 = dropped sentinel
    for c in range(NCORES):
        for w in range(WPC):
            for h in range(2):
                n = counts[c, w, h]
                if n == 0:
                    continue
                s0 = starts_flat[(c * WPC + w) * 2 + h]
                p0 = base[w, h] * 128
                idx_seq[c, p0 : p0 + n] = gl_s[s0 : s0 + n]
                slot_seq[c, p0 : p0 + n] = slot_s[s0 : s0 + n]

    # wrapped int16 gather-index layout: element j at [j%16, j//16], replicated x8
    idx16 = np.empty((NCORES, 128, TT * 8), np.int16)
    slots = np.empty((NCORES, 128, TT), ml_dtypes.bfloat16)
    for c in range(NCORES):
        a = idx_seq[c].astype(np.int16).reshape(-1, 16).T
        idx16[c] = np.tile(a, (8, 1))
        slots[c] = slot_seq[c].astype(ml_dtypes.bfloat16).reshape(TT, 128).T

    # per-core per-window dinv columns for own rows
    dcol1 = np.zeros((NCORES, 128, WPC), np.float32)
    for c in range(NCORES):
        d = np.zeros(RPAD, np.float32)
        d[:RPC] = dinv[c * RPC : (c + 1) * RPC]
        dcol1[c] = d.reshape(WPC, 128).T
    dcol2 = dcol1 * dcol1

    # static per-(window,sub) gather length: the max edge count over cores.
    # Trailing tile padding beyond it is never fetched; those message rows
    # keep stale-but-finite data that the sentinel S rows zero out.
    cmax = counts.max(axis=0).astype(np.int64)   # [WPC, 2]

    return idx16, slots, Twh, base, TT, dcol1, dcol2, dinv, cmax


def _xg_table(x, dinv):
    """dinv .* x rows in [A | B] rank-major padded order, bf16 (the L1 gather table)."""
    xd = (x * dinv[:, None]).astype(np.float32)
    xg = np.zeros((NA + NB, INC), np.float32)
    nb = RPAD - LSPL
    for rho in range(NCORES):
        xs = xd[rho * RPC : (rho + 1) * RPC]         # [6250, 256]
        xg[rho * LSPL : (rho + 1) * LSPL] = xs[:LSPL]
        xg[NA + rho * nb : NA + rho * nb + (RPC - LSPL)] = xs[LSPL:]
    return np.ascontiguousarray(xg).astype(ml_dtypes.bfloat16)


def _build(TT, Twh, base, cmax):
    nc = bacc.Bacc("TRN2", num_devices=NCORES, num_swdge_queues=4)
    f32 = mybir.dt.float32
    bf = mybir.dt.bfloat16

    xg_d = nc.dram_tensor("xg", [NA + NB, INC], bf, kind="ExternalInput")
    xo_d = nc.dram_tensor("xo", [128, WPC, INC], bf, kind="ExternalInput")
    w1b_d = nc.dram_tensor("w1b", [2, 2, 128, 128], bf, kind="ExternalInput")
    w2_d = nc.dram_tensor("w2", [2, 128, OUTC], bf, kind="ExternalInput")
    iota_d = nc.dram_tensor("iota", [128, 128], bf, kind="ExternalInput")
    ident_d = nc.dram_tensor("ident", [128, 128], bf, kind="ExternalInput")
    dc1_d = nc.dram_tensor("dcol1", [128, WPC], f32, kind="ExternalInput")
    dc2_d = nc.dram_tensor("dcol2", [128, WPC], f32, kind="ExternalInput")
    idx_d = nc.dram_tensor("idx", [128, TT * 8], mybir.dt.int16, kind="ExternalInput")
    slots_d = nc.dram_tensor("slots", [128, TT], bf, kind="ExternalInput")
    out_d = nc.dram_tensor("out", [RPAD, OUTC], f32, kind="ExternalOutput")

    # tiles per supergather (group, sub)
    Tg = np.zeros((NGRP, 2), np.int64)
    for gi in range(NGRP):
        ws = range(gi * GRP, min((gi + 1) * GRP, WPC))
        for h in range(2):
            Tg[gi, h] = sum(int(Twh[w, h]) for w in ws)

    with tile.TileContext(nc) as tc:
        nc.gpsimd.load_library(mlp)
        with (
            tc.tile_pool(name="const", bufs=1) as cpool,
            tc.tile_pool(name="gt", bufs=1) as gtpool,
            tc.tile_pool(name="evac", bufs=4) as epool,
            tc.tile_pool(name="tsp", bufs=6) as tpool,
            tc.tile_pool(name="msg", bufs=12) as mpool,
            tc.tile_pool(name="sel", bufs=8) as spool,
            tc.tile_pool(name="part", bufs=WPC) as ppool,
            # PSUM is 8 banks; every buffer costs a full bank
            tc.tile_pool(name="ps1", bufs=3, space="PSUM") as pp1,
            tc.tile_pool(name="pu", bufs=2, space="PSUM") as pu,
            tc.tile_pool(name="pt2", bufs=1, space="PSUM") as pt2,
            tc.tile_pool(name="ptr", bufs=2, space="PSUM") as ptr,
            tc.tile_pool(name="dram", bufs=1, space="DRAM") as dram,
        ):
            # ---- constants to SBUF
            w1b_s = cpool.tile([128, 2, 2, 128], bf)    # W1 as [ch_k][o_j] blocks
            w2_s = cpool.tile([128, 2, OUTC], bf)
            iota_s = cpool.tile([128, 128], bf)
            ident_s = cpool.tile([128, 128], bf)
            dc1_s = cpool.tile([128, WPC], f32)
            dc2_s = cpool.tile([128, WPC], f32)
            idx_s = cpool.tile([128, TT * 8], mybir.dt.int16)
            slots_s = cpool.tile([128, TT], bf)
            xo_s = gtpool.tile([128, WPC, INC], bf)     # own dinv.*x rows
            own2_s = gtpool.tile([128, WPC, OUTC], bf)  # own table2 rows
            nc.sync.dma_start(idx_s[:], idx_d[:])
            nc.sync.dma_start(slots_s[:], slots_d[:])
            for k in range(2):
                for j in range(2):
                    nc.sync.dma_start(w1b_s[:, k, j, :], w1b_d[k, j])
                nc.sync.dma_start(w2_s[:, k, :], w2_d[k])
            nc.sync.dma_start(iota_s[:], iota_d[:])
            nc.sync.dma_start(ident_s[:], ident_d[:])
            nc.sync.dma_start(dc1_s[:], dc1_d[:])
            nc.sync.dma_start(dc2_s[:], dc2_d[:])
            nc.scalar.dma_start(xo_s[:], xo_d[:])

            # zero the msg ring buffers once so padding-skipped rows are finite
            TMAX = int(Tg.max())
            for _ in range(12):
                mz = mpool.tile([128, TMAX, INC], bf, tag="msg")
                nc.vector.memset(mz[:], 0)

            ag2a_in = dram.tile([LSPL, OUTC], bf)
            ag2b_in = dram.tile([RPAD - LSPL, OUTC], bf)
            tb2a = dram.tile([NA, OUTC], bf)
            tb2b = dram.tile([NB, OUTC], bf)

            # ---- edge aggregation unit: gather + S build for one (group, sub)
            def gather_unit(gi, h, tbl_ap, width, qctr):
                T = int(Tg[gi, h])
                if T == 0:
                    return None, None
                ws = list(range(gi * GRP, min((gi + 1) * GRP, WPC)))
                b = int(base[ws[0], h])
                m_s = mpool.tile([128, T, width], bf, tag="msg")
                n_idx = int(cmax[gi, h]) if GRP == 1 else T * 128
                nc.gpsimd.dma_gather(
                    m_s[:], tbl_ap, idx_s[:, b * 8 : (b + T) * 8],
                    n_idx, n_idx, width,
                    single_packet=False, queue_num=qctr[0] % 4)
                qctr[0] += 1
                S_s = spool.tile([128, T, 128], bf, tag="sel")
                nc.vector.tensor_tensor(
                    out=S_s[:],
                    in0=slots_s[:, b : b + T, None].to_broadcast([128, T, 128]),
                    in1=iota_s[:, None, :].to_broadcast([128, T, 128]),
                    op=mybir.AluOpType.is_equal)
                return m_s, S_s

            def win_mms(w, h, ps, m_s, S_s, first, last):
                # base of this unit's stream is base[first window of group, h]
                gw0 = (w // GRP) * GRP
                b = int(base[gw0, h])
                n = int(Twh[w, h])
                for t in range(n):
                    tt = int(base[w, h]) - b + t
                    nc.tensor.matmul(ps[:], lhsT=S_s[:, tt, :], rhs=m_s[:, tt, :],
                                     start=(first and t == 0),
                                     stop=(last and t == n - 1))

            qctr = [0]

            # ---- layer-1 aggregation + per-window transform (B-side groups first)
            def l1_group(gi):
                ws = list(range(gi * GRP, min((gi + 1) * GRP, WPC)))
                units = {}
                for h in range(2):
                    units[h] = gather_unit(
                        gi, h, xg_d[:NA, :] if h == 0 else xg_d[NA:, :], INC, qctr)
                pss = {}
                for w in ws:
                    ps = pp1.tile([128, HID], f32, tag="ps1")
                    pss[w] = ps
                    last_h = max((h for h in range(2)
                                  if units[h][0] is not None and Twh[w, h] > 0),
                                 default=None)
                    # self-loop contribution first (operands ready at t=0)
                    nc.tensor.matmul(ps[:], lhsT=ident_s[:], rhs=xo_s[:, w, :],
                                     start=True, stop=last_h is None)
                    for h in range(2):
                        m_s, S_s = units[h]
                        if m_s is None or Twh[w, h] == 0:
                            continue
                        win_mms(w, h, ps, m_s, S_s, False, h == last_h)
                for w in ws:
                    ps = pss[w]
                    # a1 = dinv^2 .* segsum1
                    a1 = epool.tile([128, HID], bf, tag="a1")
                    nc.scalar.activation(a1[:], ps[:],
                                         mybir.ActivationFunctionType.Copy,
                                         scale=dc2_s[:, w : w + 1])
                    # a1^T via 2 PE transposes
                    a1t = tpool.tile([128, 2, 128], bf, tag="a1t")
                    for k in range(2):
                        pt = ptr.tile([128, 128], bf, tag="pt")
                        nc.tensor.transpose(pt[:], a1[:, k * 128 : (k + 1) * 128],
                                            ident_s[:])
                        nc.vector.tensor_copy(a1t[:, k, :], pt[:])
                    # u^T = (a1 @ W1)^T directly: lhsT = W1 block, rhs = a1^T
                    uT = pu.tile([128, 2, 128], f32, tag="uT")
                    for j in range(2):
                        for k in range(2):
                            nc.tensor.matmul(uT[:, j, :], lhsT=w1b_s[:, k, j, :],
                                             rhs=a1t[:, k, :],
                                             start=(k == 0), stop=(k == 1))
                    # v^T = relu(u^T) is directly the lhsT for W2
                    vt = tpool.tile([128, 2, 128], bf, tag="vt")
                    nc.scalar.activation(vt[:], uT[:],
                                         mybir.ActivationFunctionType.Relu)
                    t2_ps = pt2.tile([128, OUTC], f32, tag="pt2")
                    for k in range(2):
                        nc.tensor.matmul(t2_ps[:], lhsT=vt[:, k, :],
                                         rhs=w2_s[:, k, :],
                                         start=(k == 0), stop=(k == 1))
                    nc.vector.tensor_copy(own2_s[:, w, :], t2_ps[:])
                    if w < WA:
                        nc.sync.dma_start(ag2a_in[w * 128 : (w + 1) * 128, :],
                                          own2_s[:, w, :])
                    else:
                        nc.sync.dma_start(ag2b_in[(w - WA) * 128 : (w - WA + 1) * 128, :],
                                          own2_s[:, w, :])

            with nc.named_scope("p3_l1b"):
                for gi in GORDER[: NGRP - NGA]:
                    l1_group(gi)
            # AG2(B) launches a few groups into the A loop: the launch
            # instruction waits for the B t2-row DMAs, and the gathers queued
            # ahead of it keep the SWDGE queues draining meanwhile.
            with nc.named_scope("p3_l1a"):
                for j, gi in enumerate(GORDER[NGRP - NGA :]):
                    l1_group(gi)
                    if j == 2:
                        with nc.named_scope("ag2b"):
                            nc.gpsimd.collective_compute(
                                "AllGather", mybir.AluOpType.bypass,
                                replica_groups=[list(range(NCORES))],
                                ins=[ag2b_in.opt()], outs=[tb2b.opt()])

            # ---- layer-2 aggregation, two stages so AG latency hides
            partials = {}
            with nc.named_scope("p6_b"):
                # stage 1: self + sub-B messages -> partial (frees PSUM quickly)
                for gi in range(NGRP):
                    if gi == 5:
                        with nc.named_scope("ag2a"):
                            nc.gpsimd.collective_compute(
                                "AllGather", mybir.AluOpType.bypass,
                                replica_groups=[list(range(NCORES))],
                                ins=[ag2a_in.opt()], outs=[tb2a.opt()])
                    ws = list(range(gi * GRP, min((gi + 1) * GRP, WPC)))
                    m_s, S_s = gather_unit(gi, 1, tb2b[:, :], OUTC, qctr)
                    for w in ws:
                        ps = pp1.tile([128, OUTC], f32, tag="ps1")
                        started = False
                        if m_s is not None and Twh[w, 1] > 0:
                            win_mms(w, 1, ps, m_s, S_s, True, False)
                            started = True
                        nc.tensor.matmul(ps[:], lhsT=ident_s[:], rhs=own2_s[:, w, :],
                                         start=not started, stop=True)
                        pp = ppool.tile([128, OUTC], bf, tag="partial")
                        nc.vector.tensor_copy(pp[:], ps[:])
                        partials[w] = pp
            with nc.named_scope("p6_a"):
                # stage 2: sub-A messages + partial -> output
                for gi in range(NGRP):
                    ws = list(range(gi * GRP, min((gi + 1) * GRP, WPC)))
                    m_s, S_s = gather_unit(gi, 0, tb2a[:, :], OUTC, qctr)
                    for w in ws:
                        o_s = epool.tile([128, OUTC], f32, tag="o")
                        if m_s is not None and Twh[w, 0] > 0:
                            ps = pp1.tile([128, OUTC], f32, tag="ps1")
                            win_mms(w, 0, ps, m_s, S_s, True, True)
                            acc = epool.tile([128, OUTC], f32, tag="acc")
                            nc.vector.tensor_add(acc[:], ps[:], partials[w][:])
                        else:
                            acc = partials[w]
                        nc.scalar.activation(o_s[:], acc[:],
                                             mybir.ActivationFunctionType.Copy,
                                             scale=dc1_s[:, w : w + 1])
                        nc.sync.dma_start(out_d[w * 128 : (w + 1) * 128, :], o_s[:])

    nc.compile()
    return nc


def kernel(x, edge_index, W1, b1, W2, b2):
    x = np.asarray(x, np.float32)
    W1 = np.asarray(W1, np.float32)
    W2 = np.asarray(W2, np.float32)
    assert not np.any(np.asarray(b1)) and not np.any(np.asarray(b2)), \
        "kernel assumes zero biases (as in the reference setup)"

    idx16, slots, Twh, base, TT, dcol1, dcol2, dinv, cmax = _preprocess(np.asarray(edge_index))
    nc = _build(TT, Twh, base, cmax)

    iota = np.broadcast_to(np.arange(128, dtype=np.float32), (128, 128)).astype(ml_dtypes.bfloat16)
    ident = np.eye(128, dtype=np.float32).astype(ml_dtypes.bfloat16)
    # W1 as [ch_k, o_j] 128x128 blocks (lhsT operands for the uT matmuls)
    w1b_in = np.ascontiguousarray(
        W1.reshape(2, 128, 2, 128).transpose(0, 2, 1, 3)).astype(ml_dtypes.bfloat16)
    w2_in = np.ascontiguousarray(W2.reshape(2, 128, OUTC)).astype(ml_dtypes.bfloat16)
    xg = _xg_table(x, dinv)

    xd = (x * dinv[:, None]).astype(np.float32)
    in_maps = []
    for c in range(NCORES):
        xo = np.zeros((RPAD, INC), np.float32)
        xo[:RPC] = xd[c * RPC : (c + 1) * RPC]
        xo = np.ascontiguousarray(
            xo.reshape(WPC, 128, INC).transpose(1, 0, 2)).astype(ml_dtypes.bfloat16)
        in_maps.append({
            "xg": xg, "xo": xo,
            "w1b": w1b_in, "w2": w2_in, "iota": iota, "ident": ident,
            "dcol1": dcol1[c], "dcol2": dcol2[c],
            "idx": idx16[c], "slots": slots[c],
        })

    trace = bool(int(os.environ.get("GCN_KERNEL_TRACE", "0")))
    try:
        res = run_bass_kernel_spmd(nc, in_maps, core_ids=list(range(NCORES)), trace=trace)
    except Exception:
        # rare transient NRT exec failure: retry once on a fresh dispatch
        time_mod = __import__("time"); time_mod.sleep(2.0)
        res = run_bass_kernel_spmd(nc, in_maps, core_ids=list(range(NCORES)), trace=False)
    kernel.last_results = res
    if trace:
        print(f"HW exec time: {res.exec_time_ns} ns")
        kernel.last_exec_time_ns = res.exec_time_ns

    out = np.concatenate([res.results[c]["out"][:RPC] for c in range(NCORES)], axis=0)
    return out.astype(np.float32)


# revision 39
# speedup vs baseline: 1.4413x; 1.0072x over previous
"""GCN encoder (2-layer GCNConv, PyG-style) on 8 Trainium2 NeuronCores.

Sharding: nodes row-sharded 6250/core; edges partitioned by destination-node
owner; per-core segment-sum over 128-dst-slot windows via selection-matrix
matmuls.

Layer 1 is aggregate-then-transform: since the conv is linear before the
nonlinearity, segsum(norm .* (x@W1)[src]) == dinv_d .* segsum(dinv_s .* x[src]) @ W1,
so cores gather dinv.*x rows DIRECTLY from the (host-prepared) input table -
no replicated feature-transform GEMM, no table build on the critical path,
and gathers start at t=0. Per window w the chain is then
    a1  = dinv_d^2 .* segsum1          (scale folded into PSUM evacuation)
    t2  = relu(a1 @ W1) @ W2           (rows of the layer-2 message table)
which is exact for zero biases (as in the reference).

Layer 2: t2 is all-gathered - split into two collectives (sub-tables A/B)
that overlap with remaining gather/compute work - then aggregated the same
way; out = dinv_d .* (segsum2 + t2_own).

Self-loop messages never go through the gather path: their contribution to a
window's segment-sum is one identity matmul from an SBUF-resident copy of the
core's own rows.

Sub-tables (for int16 gather indices and collective splitting): local row
l < 3200 (windows 0-24) -> sub A (8*3200 = 25600 rows); l >= 3200
(windows 25-48) -> sub B (8*3072 = 24576 rows). Both < 2**15.
"""

import os
import numpy as np
import ml_dtypes

import concourse.bacc as bacc
import concourse.tile as tile
from concourse import bass, mybir
from concourse.bass_utils import run_bass_kernel_spmd
from concourse.library_config import mlp

N = 50000
INC, HID, OUTC = 256, 256, 128
NCORES = 8
RPC = N // NCORES            # 6250 rows per core
WPC = (RPC + 127) // 128     # 49 windows per core
RPAD = WPC * 128             # 6272
LSPL = 3200                  # sub-table split on local row (windows 0..24 | 25..48)
NA = NCORES * LSPL           # 25600 rows in sub-table A
NB = NCORES * (RPAD - LSPL)  # 24576 rows in sub-table B
WA = LSPL // 128             # 25 windows in A
GRP = 1                      # windows per supergather group
NGRP = (WPC + GRP - 1) // GRP
# L1 processes B-side groups first so AG2(B) can launch early.
# group NGA = WA//GRP straddles the A/B boundary (windows 24,25) and is
# processed in the B phase, so after the B phase windows 24..48 are all done.
NGA = WA // GRP
GORDER = list(range(NGA, NGRP)) + list(range(0, NGA))


def _preprocess(edge_index):
    """Edge partitioning / ordering and normalization constants (host, index-only)."""
    src = np.asarray(edge_index[0], np.int64)
    dst = np.asarray(edge_index[1], np.int64)

    # degrees include the self-loops the reference adds
    deg = (np.bincount(dst, minlength=N) + 1).astype(np.float64)
    dinv = (1.0 / np.sqrt(deg)).astype(np.float32)

    owner = dst // RPC
    dstl = dst - owner * RPC
    win = dstl >> 7
    slot = dstl & 127
    srho = src // RPC
    srl = src - srho * RPC
    sub = (srl >= LSPL).astype(np.int64)
    gl = np.where(sub == 0, srho * LSPL + srl,
                  srho * (RPAD - LSPL) + (srl - LSPL)).astype(np.int32)

    key = (owner * WPC + win) * 2 + sub
    order = np.argsort(key, kind="stable")
    key_s = key[order]
    gl_s = gl[order]
    slot_s = slot[order].astype(np.int32)

    nbuck = NCORES * WPC * 2
    counts = np.bincount(key_s, minlength=nbuck).reshape(NCORES, WPC, 2)
    starts_flat = np.concatenate([[0], np.cumsum(counts.reshape(-1))])

    # tiles per (window, sub): max over cores so one SPMD program fits all
    Twh = (counts.max(axis=0) + 127) // 128     # [WPC, 2]
    TT = int(Twh.sum())
    # stream order: group -> sub -> window in group -> tiles
    base = np.zeros((WPC, 2), np.int64)
    pos = 0
    for gi in range(NGRP):
        ws = range(gi * GRP, min((gi + 1) * GRP, WPC))
        for h in range(2):
            for w in ws:
                base[w, h] = pos
                pos += Twh[w, h]
    assert pos == TT

    idx_seq = np.zeros((NCORES, TT * 128), np.int32)
    slot_seq = np.full((NCORES, TT * 128), 128, np.int32)  # 128 = dropped sentinel
    for c in range(NCORES):
        for w in range(WPC):
            for h in range(2):
                n = counts[c, w, h]
                if n == 0:
                    continue
                s0 = starts_flat[(c * WPC + w) * 2 + h]
                p0 = base[w, h] * 128
                idx_seq[c, p0 : p0 + n] = gl_s[s0 : s0 + n]
                slot_seq[c, p0 : p0 + n] = slot_s[s0 : s0 + n]

    # wrapped int16 gather-index layout: element j at [j%16, j//16], replicated x8
    idx16 = np.empty((NCORES, 128, TT * 8), np.int16)
    slots = np.empty((NCORES, 128, TT), ml_dtypes.bfloat16)
    for c in range(NCORES):
        a = idx_seq[c].astype(np.int16).reshape(-1, 16).T
        idx16[c] = np.tile(a, (8, 1))
        slots[c] = slot_seq[c].astype(ml_dtypes.bfloat16).reshape(TT, 128).T

    # per-core per-window dinv columns for own rows
    dcol1 = np.zeros((NCORES, 128, WPC), np.float32)
    for c in range(NCORES):
        d = np.zeros(RPAD, np.float32)
        d[:RPC] = dinv[c * RPC : (c + 1) * RPC]
        dcol1[c] = d.reshape(WPC, 128).T
    dcol2 = dcol1 * dcol1

    # static per-(window,sub) gather length: the max edge count over cores.
    # Trailing tile padding beyond it is never fetched; those message rows
    # keep stale-but-finite data that the sentinel S rows zero out.
    cmax = counts.max(axis=0).astype(np.int64)   # [WPC, 2]

    return idx16, slots, Twh, base, TT, dcol1, dcol2, dinv, cmax


def _xg_table(x, dinv):
    """dinv .* x rows in [A | B] rank-major padded order, bf16 (the L1 gather table)."""
    xd = (x * dinv[:, None]).astype(np.float32)
    xg = np.zeros((NA + NB, INC), np.float32)
    nb = RPAD - LSPL
    for rho in range(NCORES):
        xs = xd[rho * RPC : (rho + 1) * RPC]         # [6250, 256]
        xg[rho * LSPL : (rho + 1) * LSPL] = xs[:LSPL]
        xg[NA + rho * nb : NA + rho * nb + (RPC - LSPL)] = xs[LSPL:]
    return np.ascontiguousarray(xg).astype(ml_dtypes.bfloat16)


def _build(TT, Twh, base, cmax):
    nc = bacc.Bacc("TRN2", num_devices=NCORES, num_swdge_queues=4)
    f32 = mybir.dt.float32
    bf = mybir.dt.bfloat16

    xg_d = nc.dram_tensor("xg", [NA + NB, INC], bf, kind="ExternalInput")
    xo_d = nc.dram_tensor("xo", [128, WPC, INC], bf, kind="ExternalInput")
    w1b_d = nc.dram_tensor("w1b", [2, 2, 128, 128], bf, kind="ExternalInput")
    w2_d = nc.dram_tensor("w2", [2, 128, OUTC], bf, kind="ExternalInput")
    iota_d = nc.dram_tensor("iota", [128, 128], bf, kind="ExternalInput")
    ident_d = nc.dram_tensor("ident", [128, 128], bf, kind="ExternalInput")
    dc1_d = nc.dram_tensor("dcol1", [128, WPC], f32, kind="ExternalInput")
    dc2_d = nc.dram_tensor("dcol2", [128, WPC], f32, kind="ExternalInput")
    idx_d = nc.dram_tensor("idx", [128, TT * 8], mybir.dt.int16, kind="ExternalInput")
    slots_d = nc.dram_tensor("slots", [128, TT], bf, kind="ExternalInput")
    out_d = nc.dram_tensor("out", [RPAD, OUTC], f32, kind="ExternalOutput")

    # tiles per supergather (group, sub)
    Tg = np.zeros((NGRP, 2), np.int64)
    for gi in range(NGRP):
        ws = range(gi * GRP, min((gi + 1) * GRP, WPC))
        for h in range(2):
            Tg[gi, h] = sum(int(Twh[w, h]) for w in ws)

    with tile.TileContext(nc) as tc:
        nc.gpsimd.load_library(mlp)
        with (
            tc.tile_pool(name="const", bufs=1) as cpool,
            tc.tile_pool(name="gt", bufs=1) as gtpool,
            tc.tile_pool(name="evac", bufs=4) as epool,
            tc.tile_pool(name="tsp", bufs=6) as tpool,
            tc.tile_pool(name="msg", bufs=12) as mpool,
            tc.tile_pool(name="sel", bufs=8) as spool,
            tc.tile_pool(name="part", bufs=WPC) as ppool,
            # PSUM is 8 banks; every buffer costs a full bank
            tc.tile_pool(name="ps1", bufs=3, space="PSUM") as pp1,
            tc.tile_pool(name="pu", bufs=2, space="PSUM") as pu,
            tc.tile_pool(name="pt2", bufs=1, space="PSUM") as pt2,
            tc.tile_pool(name="ptr", bufs=2, space="PSUM") as ptr,
            tc.tile_pool(name="dram", bufs=1, space="DRAM") as dram,
        ):
            # ---- constants to SBUF
            w1b_s = cpool.tile([128, 2, 2, 128], bf)    # W1 as [ch_k][o_j] blocks
            w2_s = cpool.tile([128, 2, OUTC], bf)
            iota_s = cpool.tile([128, 128], bf)
            ident_s = cpool.tile([128, 128], bf)
            dc1_s = cpool.tile([128, WPC], f32)
            dc2_s = cpool.tile([128, WPC], f32)
            idx_s = cpool.tile([128, TT * 8], mybir.dt.int16)
            slots_s = cpool.tile([128, TT], bf)
            xo_s = gtpool.tile([128, WPC, INC], bf)     # own dinv.*x rows
            own2_s = gtpool.tile([128, WPC, OUTC], bf)  # own table2 rows
            nc.sync.dma_start(idx_s[:], idx_d[:])
            nc.sync.dma_start(slots_s[:], slots_d[:])
            for k in range(2):
                for j in range(2):
                    nc.sync.dma_start(w1b_s[:, k, j, :], w1b_d[k, j])
                nc.sync.dma_start(w2_s[:, k, :], w2_d[k])
            nc.sync.dma_start(iota_s[:], iota_d[:])
            nc.sync.dma_start(ident_s[:], ident_d[:])
            nc.sync.dma_start(dc1_s[:], dc1_d[:])
            nc.sync.dma_start(dc2_s[:], dc2_d[:])
            nc.scalar.dma_start(xo_s[:], xo_d[:])

            # zero the msg ring buffers once so padding-skipped rows are finite
            TMAX = int(Tg.max())
            for _ in range(12):
                mz = mpool.tile([128, TMAX, INC], bf, tag="msg")
                nc.vector.memset(mz[:], 0)

            ag2a_in = dram.tile([LSPL, OUTC], bf)
            ag2b_in = dram.tile([RPAD - LSPL, OUTC], bf)
            tb2a = dram.tile([NA, OUTC], bf)
            tb2b = dram.tile([NB, OUTC], bf)

            # ---- edge aggregation unit: gather + S build for one (group, sub)
            def gather_unit(gi, h, tbl_ap, width, qctr):
                T = int(Tg[gi, h])
                if T == 0:
                    return None, None
                ws = list(range(gi * GRP, min((gi + 1) * GRP, WPC)))
                b = int(base[ws[0], h])
                m_s = mpool.tile([128, T, width], bf, tag="msg")
                n_idx = int(cmax[gi, h]) if GRP == 1 else T * 128
                nc.gpsimd.dma_gather(
                    m_s[:], tbl_ap, idx_s[:, b * 8 : (b + T) * 8],
                    n_idx, n_idx, width,
                    single_packet=False, queue_num=qctr[0] % 4)
                qctr[0] += 1
                S_s = spool.tile([128, T, 128], bf, tag="sel")
                nc.vector.tensor_tensor(
                    out=S_s[:],
                    in0=slots_s[:, b : b + T, None].to_broadcast([128, T, 128]),
                    in1=iota_s[:, None, :].to_broadcast([128, T, 128]),
                    op=mybir.AluOpType.is_equal)
                return m_s, S_s

            def win_mms(w, h, ps, m_s, S_s, first, last):
                # base of this unit's stream is base[first window of group, h]
                gw0 = (w // GRP) * GRP
                b = int(base[gw0, h])
                n = int(Twh[w, h])
                for t in range(n):
                    tt = int(base[w, h]) - b + t
                    nc.tensor.matmul(ps[:], lhsT=S_s[:, tt, :], rhs=m_s[:, tt, :],
                                     start=(first and t == 0),
                                     stop=(last and t == n - 1))

            qctr = [0]

            # ---- layer-1 aggregation + per-window transform (B-side groups first)
            def l1_group(gi):
                ws = list(range(gi * GRP, min((gi + 1) * GRP, WPC)))
                units = {}
                for h in range(2):
                    units[h] = gather_unit(
                        gi, h, xg_d[:NA, :] if h == 0 else xg_d[NA:, :], INC, qctr)
                pss = {}
                for w in ws:
                    ps = pp1.tile([128, HID], f32, tag="ps1")
                    pss[w] = ps
                    last_h = max((h for h in range(2)
                                  if units[h][0] is not None and Twh[w, h] > 0),
                                 default=None)
                    # self-loop contribution first (operands ready at t=0)
                    nc.tensor.matmul(ps[:], lhsT=ident_s[:], rhs=xo_s[:, w, :],
                                     start=True, stop=last_h is None)
                    for h in range(2):
                        m_s, S_s = units[h]
                        if m_s is None or Twh[w, h] == 0:
                            continue
                        win_mms(w, h, ps, m_s, S_s, False, h == last_h)
                for w in ws:
                    ps = pss[w]
                    # a1 = dinv^2 .* segsum1
                    a1 = epool.tile([128, HID], bf, tag="a1")
                    nc.scalar.activation(a1[:], ps[:],
                                         mybir.ActivationFunctionType.Copy,
                                         scale=dc2_s[:, w : w + 1])
                    # a1^T via 2 PE transposes
                    a1t = tpool.tile([128, 2, 128], bf, tag="a1t")
                    for k in range(2):
                        pt = ptr.tile([128, 128], bf, tag="pt")
                        nc.tensor.transpose(pt[:], a1[:, k * 128 : (k + 1) * 128],
                                            ident_s[:])
                        nc.vector.tensor_copy(a1t[:, k, :], pt[:])
                    # u^T = (a1 @ W1)^T directly: lhsT = W1 block, rhs = a1^T
                    uT = pu.tile([128, 2, 128], f32, tag="uT")
                    for j in range(2):
                        for k in range(2):
                            nc.tensor.matmul(uT[:, j, :], lhsT=w1b_s[:, k, j, :],
                                             rhs=a1t[:, k, :],
                                             start=(k == 0), stop=(k == 1))
                    # v^T = relu(u^T) is directly the lhsT for W2
                    vt = tpool.tile([128, 2, 128], bf, tag="vt")
                    nc.scalar.activation(vt[:], uT[:],
                                         mybir.ActivationFunctionType.Relu)
                    t2_ps = pt2.tile([128, OUTC], f32, tag="pt2")
                    for k in range(2):
                        nc.tensor.matmul(t2_ps[:], lhsT=vt[:, k, :],
                                         rhs=w2_s[:, k, :],
                                         start=(k == 0), stop=(k == 1))
                    nc.vector.tensor_copy(own2_s[:, w, :], t2_ps[:])
                    if w < WA:
                        nc.sync.dma_start(ag2a_in[w * 128 : (w + 1) * 128, :],
                                          own2_s[:, w, :])
                    else:
                        nc.sync.dma_start(ag2b_in[(w - WA) * 128 : (w - WA + 1) * 128, :],
                                          own2_s[:, w, :])

            with nc.named_scope("p3_l1b"):
                for gi in GORDER[: NGRP - NGA]:
                    l1_group(gi)
            # AG2(B) launches a few groups into the A loop: the launch
            # instruction waits for the B t2-row DMAs, and the gathers queued
            # ahead of it keep the SWDGE queues draining meanwhile.
            with nc.named_scope("p3_l1a"):
                for j, gi in enumerate(GORDER[NGRP - NGA :]):
                    l1_group(gi)
                    if j == 1:
                        with nc.named_scope("ag2b"):
                            nc.gpsimd.collective_compute(
                                "AllGather", mybir.AluOpType.bypass,
                                replica_groups=[list(range(NCORES))],
                                ins=[ag2b_in.opt()], outs=[tb2b.opt()])
            # AG2(A) queues on the CC engine right behind AG2(B); its input is
            # complete once the A-side windows drain, and p6_b's first gathers
            # wait on tb2b anyway, so this launch costs the Pool queue nothing.
            with nc.named_scope("ag2a"):
                nc.gpsimd.collective_compute(
                    "AllGather", mybir.AluOpType.bypass,
                    replica_groups=[list(range(NCORES))],
                    ins=[ag2a_in.opt()], outs=[tb2a.opt()])

            # ---- layer-2 aggregation, two stages so AG latency hides
            partials = {}
            with nc.named_scope("p6_b"):
                # stage 1: self + sub-B messages -> partial (frees PSUM quickly)
                for gi in range(NGRP):
                    ws = list(range(gi * GRP, min((gi + 1) * GRP, WPC)))
                    m_s, S_s = gather_unit(gi, 1, tb2b[:, :], OUTC, qctr)
                    for w in ws:
                        ps = pp1.tile([128, OUTC], f32, tag="ps1")
                        started = False
                        if m_s is not None and Twh[w, 1] > 0:
                            win_mms(w, 1, ps, m_s, S_s, True, False)
                            started = True
                        nc.tensor.matmul(ps[:], lhsT=ident_s[:], rhs=own2_s[:, w, :],
                                         start=not started, stop=True)
                        pp = ppool.tile([128, OUTC], bf, tag="partial")
                        nc.vector.tensor_copy(pp[:], ps[:])
                        partials[w] = pp
            with nc.named_scope("p6_a"):
                # stage 2: sub-A messages + partial -> output
                for gi in range(NGRP):
                    ws = list(range(gi * GRP, min((gi + 1) * GRP, WPC)))
                    m_s, S_s = gather_unit(gi, 0, tb2a[:, :], OUTC, qctr)
                    for w in ws:
                        o_s = epool.tile([128, OUTC], f32, tag="o")
                        if m_s is not None and Twh[w, 0] > 0:
                            ps = pp1.tile([128, OUTC], f32, tag="ps1")
                            win_mms(w, 0, ps, m_s, S_s, True, True)
                            acc = epool.tile([128, OUTC], f32, tag="acc")
                            nc.vector.tensor_add(acc[:], ps[:], partials[w][:])
                        else:
                            acc = partials[w]
                        nc.scalar.activation(o_s[:], acc[:],
                                             mybir.ActivationFunctionType.Copy,
                                             scale=dc1_s[:, w : w + 1])
                        nc.sync.dma_start(out_d[w * 128 : (w + 1) * 128, :], o_s[:])

    nc.compile()
    return nc


def kernel(x, edge_index, W1, b1, W2, b2):
    x = np.asarray(x, np.float32)
    W1 = np.asarray(W1, np.float32)
    W2 = np.asarray(W2, np.float32)
    assert not np.any(np.asarray(b1)) and not np.any(np.asarray(b2)), \
        "kernel assumes zero biases (as in the reference setup)"

    idx16, slots, Twh, base, TT, dcol1, dcol2, dinv, cmax = _preprocess(np.asarray(edge_index))
    nc = _build(TT, Twh, base, cmax)

    iota = np.broadcast_to(np.arange(128, dtype=np.float32), (128, 128)).astype(ml_dtypes.bfloat16)
    ident = np.eye(128, dtype=np.float32).astype(ml_dtypes.bfloat16)
    # W1 as [ch_k, o_j] 128x128 blocks (lhsT operands for the uT matmuls)
    w1b_in = np.ascontiguousarray(
        W1.reshape(2, 128, 2, 128).transpose(0, 2, 1, 3)).astype(ml_dtypes.bfloat16)
    w2_in = np.ascontiguousarray(W2.reshape(2, 128, OUTC)).astype(ml_dtypes.bfloat16)
    xg = _xg_table(x, dinv)

    xd = (x * dinv[:, None]).astype(np.float32)
    in_maps = []
    for c in range(NCORES):
        xo = np.zeros((RPAD, INC), np.float32)
        xo[:RPC] = xd[c * RPC : (c + 1) * RPC]
        xo = np.ascontiguousarray(
            xo.reshape(WPC, 128, INC).transpose(1, 0, 2)).astype(ml_dtypes.bfloat16)
        in_maps.append({
            "xg": xg, "xo": xo,
            "w1b": w1b_in, "w2": w2_in, "iota": iota, "ident": ident,
            "dcol1": dcol1[c], "dcol2": dcol2[c],
            "idx": idx16[c], "slots": slots[c],
        })

    trace = bool(int(os.environ.get("GCN_KERNEL_TRACE", "0")))
    try:
        res = run_bass_kernel_spmd(nc, in_maps, core_ids=list(range(NCORES)), trace=trace)
    except Exception:
        # rare transient NRT exec failure: retry once on a fresh dispatch
        time_mod = __import__("time"); time_mod.sleep(2.0)
        res = run_bass_kernel_spmd(nc, in_maps, core_ids=list(range(NCORES)), trace=False)
    kernel.last_results = res
    if trace:
        print(f"HW exec time: {res.exec_time_ns} ns")
        kernel.last_exec_time_ns = res.exec_time_ns

    out = np.concatenate([res.results[c]["out"][:RPC] for c in range(NCORES)], axis=0)
    return out.astype(np.float32)
